# revision 19
# baseline (speedup 1.0000x reference)
"""Multi-head attention Trainium2 kernel (8 NeuronCores, Bass/Tile).

Sharding: core c -> (batch b = c//2, head-group hg = c%2). Each core computes
attention for 8 of the 16 heads of one batch element plus its partial
out-projection; the host sums the two head-group partials per batch.

Per-core layouts (host pre-transposes inputs; contraction dims on partitions):
  xT  [E=1024, S=2048]      x[b].T
  wqT/wkT/wvT [1024, 512]   W[hg_rows].T
  woT [512, 1024]           Wo[:, hg_cols].T
  sel [8, 512]              0/1 selector for softmax-denominator replication

On-chip pipeline (all fp32):
  QT = wqT.T-tiles @ xT   [512, 2048] (head-major, transposed)
  KT likewise; V natural [2048, 512] with a ones-column appended per head
  scoresT[t,s] = KT_h.T-tile @ QT_h  (K=64, two heads row-packed per PE pass)
  expT = exp(scoresT/8) on ScalarE, batched [128, 2048] over 4 psum banks
  (outT | Z) = [V_h | 1].T @ expT    (M=65 matmul: row 64 = softmax sums)
  outT_norm = outT * replicate(1/Z)  (K=8 selector matmul + DVE mult)
  out = outT_norm.T-tiles @ woT      [2048, 1024] partial
"""

import os
import sys
import types

import numpy as np

B, S, E, H = 4, 2048, 1024, 16
DK = E // H  # 64
HG = H // 2  # heads per core = 8
DG = HG * DK  # 512 projected dims per core
NCORES = 8

TRACE = bool(os.environ.get("TRN_KERNEL_TRACE"))
# matmul-operand dtype: bf16 single-pass PE (fp32 PSUM accumulation) vs
# fp32 operands (PE double-pumps each matmul -> ~2x slower)
MM_DTYPE = os.environ.get("TRN_MM_DTYPE", "bf16")
LAST_EXEC_TIME_NS = None

_cache = {}


def _env_setup():
    import antenv

    if "antenv.axon_hooks" not in sys.modules:
        mod = types.ModuleType("antenv.axon_hooks")
        mod._hook = None
        mod.set_axon_ntff_profile_hook = lambda h: setattr(mod, "_hook", h)
        mod.get_axon_ntff_profile_hook = lambda: mod._hook
        sys.modules["antenv.axon_hooks"] = mod
        antenv.axon_hooks = mod
        try:
            from trn_agent_boot.trn_boot import _ntff_profile_via_ctypes

            mod.set_axon_ntff_profile_hook(
                _ntff_profile_via_ctypes("/opt/axon/libaxon_pjrt.so")
            )
        except Exception:
            pass

    import concourse.bass_utils as bass_utils

    bass_utils.upload_artifacts = lambda tmpdir: tmpdir

    import concourse.tile as tile
    from concourse import mybir
    from concourse.vector_clock import ScopedClock

    if getattr(tile.TileContext, "_wait_split_patched", False):
        return

    MAX_WAITS = 1  # walrus on this image rejects >1 sync wait per instruction

    def _drain_and_barrier_split(self, tick_clock, wait_clock):
        probe = self.nc.sync.drain()
        wait_clock.add_sem_waits(
            probe.ins, ScopedClock({None: tick_clock.global_clock})
        )
        waits = list(probe.ins.sync_info.on_wait)
        if len(waits) > MAX_WAITS:
            num2h = {h.num: h for h in self.sems.allocated().values()}
            probe.ins.sync_info.on_wait = []
            for w in waits:
                self.nc.sync.wait_ge(num2h[w.id], w.wait_value)
            self.nc.sync.drain()
        self.nc.all_engine_barrier()
        popped = self.nc._tile_sem_poison_stack.pop()
        assert popped is self._sem_poison
        self.nc.clear_and_free_semaphores(list(self.sems.allocated().values()))
        self.nc.all_engine_barrier()

    _orig_commit = tile.TileContext._commit_instruction
    _ctr = [0]

    def _commit_split_waits(self, inst, lazy_reg_writes=True):
        si = inst.sync_info
        if (
            si is not None
            and len(si.on_wait) > MAX_WAITS
            and inst.engine != mybir.EngineType.Unassigned
        ):
            waits = list(si.on_wait)
            keep, hoist = waits[:MAX_WAITS], waits[MAX_WAITS:]
            for i in range(0, len(hoist), MAX_WAITS):
                _ctr[0] += 1
                nop = mybir.InstNoOp(name=f"waitnop-{_ctr[0]}", ins=[], outs=[])
                nop.engine = inst.engine
                nop.sync_info = mybir.SyncInfo(
                    on_wait=hoist[i : i + MAX_WAITS], on_update=[]
                )
                self.nc.register_instruction(nop, overwrite=True)
                _orig_commit(self, nop, lazy_reg_writes=False)
            inst.sync_info = mybir.SyncInfo(on_wait=keep, on_update=list(si.on_update))
        return _orig_commit(self, inst, lazy_reg_writes=lazy_reg_writes)

    tile.TileContext._drain_and_barrier = _drain_and_barrier_split
    tile.TileContext._commit_instruction = _commit_split_waits
    tile.TileContext._wait_split_patched = True

    # use the full usable SBUF on trn2 (default constant is stale)
    import concourse.tile_utils as tile_utils

    tile_utils.max_sbuf_usage = 206 * 1024


def _build_nc():
    import contextlib

    import concourse.bass as bass
    import concourse.tile as tile
    from concourse import mybir

    F32 = mybir.dt.float32
    CDT = mybir.dt.bfloat16 if MM_DTYPE == "bf16" else mybir.dt.float32
    PS = bass.MemorySpace.PSUM
    AF = mybir.ActivationFunctionType

    nc = bass.Bass()
    xT_d = nc.dram_tensor("xT", [E, S], CDT, kind="ExternalInput")
    wqT_d = nc.dram_tensor("wqT", [E, DG], CDT, kind="ExternalInput")
    wkT_d = nc.dram_tensor("wkT", [E, DG], CDT, kind="ExternalInput")
    wvT_d = nc.dram_tensor("wvT", [E, DG], CDT, kind="ExternalInput")
    woT_d = nc.dram_tensor("woT", [DG, E], CDT, kind="ExternalInput")
    sel_d = nc.dram_tensor("sel", [HG, 512], F32, kind="ExternalInput")
    out_d = nc.dram_tensor("out", [S, E], F32, kind="ExternalOutput")

    NE = E // 128  # 8 e-tiles
    NT = S // 128  # 16 t/s-tiles
    NNC = S // 512  # 4 s-chunks
    NM = DG // 128  # 4 head-pair tiles

    with tile.TileContext(nc) as tc:
        with tc.tile_pool(name="persist", bufs=1) as pp:
            sel_sb = pp.tile([HG, 512], F32, tag="sel")
            nc.sync.dma_start(sel_sb[:], sel_d[:])

            QT = pp.tile([128, NM * S], CDT, tag="QT")  # [128, 8192]
            KT = pp.tile([128, NM * S], CDT, tag="KT")
            Vsb = pp.tile([128, NT * DG], CDT, tag="V")  # [128, 8192]
            ones = pp.tile([128, 1], CDT, tag="ones")
            nc.gpsimd.memset(ones[:], 1.0)

            # ---- projections (scoped psum + x/weight pools free afterwards) --
            with (
                tc.tile_pool(name="projps", bufs=3, space=PS) as proj_ps,
                tc.tile_pool(name="xtp", bufs=1) as xtp,
            ):
                xT = xtp.tile([128, NE * S], CDT, tag="xT")  # [128, 16384]
                for j in range(NE):
                    nc.sync.dma_start(
                        xT[:, j * S : (j + 1) * S], xT_d[j * 128 : (j + 1) * 128, :]
                    )

                for name, w_d, dst in (("q", wqT_d, QT), ("k", wkT_d, KT)):
                    with tc.tile_pool(name=f"w{name}", bufs=1) as wp:
                        wT = wp.tile([128, NE * DG], CDT, tag=f"w{name}T")
                        for j in range(NE):
                            nc.sync.dma_start(
                                wT[:, j * DG : (j + 1) * DG],
                                w_d[j * 128 : (j + 1) * 128, :],
                            )
                        for m in range(NM):
                            for n in range(NNC):
                                acc = proj_ps.tile([128, 512], F32, tag="acc")
                                for j in range(NE):
                                    nc.tensor.matmul(
                                        acc[:],
                                        wT[:, j * DG + m * 128 : j * DG + (m + 1) * 128],
                                        xT[:, j * S + n * 512 : j * S + (n + 1) * 512],
                                        start=(j == 0),
                                        stop=(j == NE - 1),
                                    )
                                nc.vector.tensor_copy(
                                    dst[:, m * S + n * 512 : m * S + (n + 1) * 512],
                                    acc[:],
                                )

                with tc.tile_pool(name="wv", bufs=1) as wp:
                    wvT = wp.tile([128, NE * DG], CDT, tag="wvT")
                    for j in range(NE):
                        nc.sync.dma_start(
                            wvT[:, j * DG : (j + 1) * DG],
                            wvT_d[j * 128 : (j + 1) * 128, :],
                        )
                    for i in range(NT):
                        acc = proj_ps.tile([128, 512], F32, tag="acc")
                        for j in range(NE):
                            nc.tensor.matmul(
                                acc[:],
                                xT[:, j * S + i * 128 : j * S + (i + 1) * 128],
                                wvT[:, j * DG : (j + 1) * DG],
                                start=(j == 0),
                                stop=(j == NE - 1),
                            )
                        nc.vector.tensor_copy(Vsb[:, i * DG : (i + 1) * DG], acc[:])

            # ---- attention phase pools (reuse freed SBUF/PSUM) ----
            attn_stack = contextlib.ExitStack()
            pp2 = attn_stack.enter_context(tc.tile_pool(name="persist2", bufs=1))
            onorm = pp2.tile([128, NM * S], CDT, tag="onorm")
            zbuf = pp2.tile([HG, S], F32, tag="zbuf")
            zrec = pp2.tile([HG, S], F32, tag="zrec")
            zstage = pp2.tile([128, NM * 512], F32, tag="zstage")
            woT = pp2.tile([128, NM * E], CDT, tag="woT")  # [128, 4096]
            for k in range(NM):
                nc.sync.dma_start(
                    woT[:, k * E : (k + 1) * E], woT_d[k * 128 : (k + 1) * 128, :]
                )
            stg = attn_stack.enter_context(tc.tile_pool(name="stage", bufs=6))
            expp = attn_stack.enter_context(tc.tile_pool(name="expp", bufs=4))
            outp = attn_stack.enter_context(tc.tile_pool(name="outp", bufs=2))
            sc_ps = attn_stack.enter_context(
                tc.tile_pool(name="scpsum", bufs=2, space=PS)
            )
            av_ps = attn_stack.enter_context(
                tc.tile_pool(name="avpsum", bufs=2, space=PS)
            )
            z_ps = attn_stack.enter_context(tc.tile_pool(name="zpsum", bufs=1, space=PS))
            ms_ps = attn_stack.enter_context(
                tc.tile_pool(name="miscpsum", bufs=1, space=PS)
            )

            def attn_step(hp, n, t, av, zz, zrow):
                """scores -> exp -> AV pair + denominator pair for one t-tile."""
                sc = sc_ps.tile([128, 1024], F32, tag="sc")
                nc.tensor.matmul(
                    sc[:, 0:512],
                    KT[0:64, hp * S + t * 128 : hp * S + (t + 1) * 128],
                    QT[0:64, hp * S + n * 512 : hp * S + (n + 1) * 512],
                )
                nc.tensor.matmul(
                    sc[:, 512:1024],
                    KT[64:128, hp * S + t * 128 : hp * S + (t + 1) * 128],
                    QT[64:128, hp * S + n * 512 : hp * S + (n + 1) * 512],
                )
                ex = expp.tile([128, 1024], CDT, tag="ex")
                nc.scalar.activation(ex[:], sc[:], AF.Exp, scale=0.125)
                voff = t * DG
                nc.tensor.matmul(
                    av[0:64, :],
                    Vsb[:, voff + (2 * hp) * DK : voff + (2 * hp) * DK + DK],
                    ex[:, 0:512],
                    start=(t == 0),
                    stop=(t == NT - 1),
                    tile_position=(0, 0),
                    skip_group_check=True,
                )
                nc.tensor.matmul(
                    av[64:128, :],
                    Vsb[:, voff + (2 * hp + 1) * DK : voff + (2 * hp + 1) * DK + DK],
                    ex[:, 512:1024],
                    start=(t == 0),
                    stop=(t == NT - 1),
                    tile_position=(0, 64),
                    skip_group_check=True,
                )
                nc.tensor.matmul(
                    zz[zrow : zrow + 1, :],
                    ones[:, 0:1],
                    ex[:, 0:512],
                    start=(t == 0),
                    stop=(t == NT - 1),
                    tile_position=(0, zrow),
                    skip_group_check=True,
                )
                nc.tensor.matmul(
                    zz[zrow + 32 : zrow + 33, :],
                    ones[:, 0:1],
                    ex[:, 512:1024],
                    start=(t == 0),
                    stop=(t == NT - 1),
                    tile_position=(0, zrow + 32),
                    skip_group_check=True,
                )

            for n in range(NNC):
                uos = {}
                # two interleaved head-pair streams hide each other's waits
                for hpp in (0, 2):
                    av_a = av_ps.tile([128, 512], F32, tag="av")
                    av_b = av_ps.tile([128, 512], F32, tag="av")
                    zz = z_ps.tile([128, 512], F32, tag="zz")
                    for t in range(NT):
                        attn_step(hpp, n, t, av_a, zz, 0)
                        attn_step(hpp + 1, n, t, av_b, zz, 64)
                    for hp, av, zrow in ((hpp, av_a, 0), (hpp + 1, av_b, 64)):
                        uo = stg.tile([128, 512], F32, tag="uo")
                        nc.vector.tensor_copy(uo[:], av[:])
                        nc.vector.tensor_copy(
                            zstage[0:1, hp * 512 : (hp + 1) * 512],
                            zz[zrow : zrow + 1, :],
                        )
                        nc.vector.tensor_copy(
                            zstage[32:33, hp * 512 : (hp + 1) * 512],
                            zz[zrow + 32 : zrow + 33, :],
                        )
                        uos[hp] = uo

                # zbuf rows 0-3 = even heads of pair hp, rows 4-7 = odd heads
                for hp in range(NM):
                    nc.sync.dma_start(
                        zbuf[hp : hp + 1, n * 512 : (n + 1) * 512],
                        zstage[0:1, hp * 512 : (hp + 1) * 512],
                    )
                    nc.sync.dma_start(
                        zbuf[4 + hp : 5 + hp, n * 512 : (n + 1) * 512],
                        zstage[32:33, hp * 512 : (hp + 1) * 512],
                    )
                nc.vector.reciprocal(
                    zrec[:, n * 512 : (n + 1) * 512], zbuf[:, n * 512 : (n + 1) * 512]
                )
                for k in range(NM):
                    rep = ms_ps.tile([128, 512], F32, tag="ms")
                    nc.tensor.matmul(
                        rep[:],
                        sel_sb[:, k * 128 : (k + 1) * 128],
                        zrec[:, n * 512 : (n + 1) * 512],
                    )
                    nc.vector.tensor_tensor(
                        onorm[:, k * S + n * 512 : k * S + (n + 1) * 512],
                        uos[k][:],
                        rep[:],
                        mybir.AluOpType.mult,
                    )

                for i in range(4 * n, 4 * n + 4):
                    osb = outp.tile([128, E], F32, tag="osb")
                    for eh in range(2):
                        ps = ms_ps.tile([128, 512], F32, tag="ms")
                        for k in range(NM):
                            nc.tensor.matmul(
                                ps[:],
                                onorm[:, k * S + i * 128 : k * S + (i + 1) * 128],
                                woT[:, k * E + eh * 512 : k * E + (eh + 1) * 512],
                                start=(k == 0),
                                stop=(k == NM - 1),
                            )
                        nc.vector.tensor_copy(osb[:, eh * 512 : (eh + 1) * 512], ps[:])
                    nc.sync.dma_start(out_d[i * 128 : (i + 1) * 128, :], osb[:])

            attn_stack.close()

    return nc


def _make_sel():
    # zbuf row for head (2k + p//64): even heads -> row k, odd heads -> row 4+k
    sel = np.zeros((HG, 512), dtype=np.float32)
    for k in range(4):
        for p in range(128):
            r = k if p < 64 else 4 + k
            sel[r, k * 128 + p] = 1.0
    return sel


def kernel(x, Wq, Wk, Wv, Wo):
    global LAST_EXEC_TIME_NS
    _env_setup()
    from concourse.bass_utils import run_bass_kernel_spmd

    x = np.asarray(x, dtype=np.float32)
    Wq = np.asarray(Wq, dtype=np.float32)
    Wk = np.asarray(Wk, dtype=np.float32)
    Wv = np.asarray(Wv, dtype=np.float32)
    Wo = np.asarray(Wo, dtype=np.float32)

    if "nc" not in _cache:
        _cache["nc"] = _build_nc()
    nc = _cache["nc"]

    if MM_DTYPE == "bf16":
        import ml_dtypes

        cdt = ml_dtypes.bfloat16
    else:
        cdt = np.float32

    sel = _make_sel()
    in_maps = []
    for c in range(NCORES):
        b, hg = c // 2, c % 2
        r = slice(hg * DG, (hg + 1) * DG)
        in_maps.append(
            {
                "xT": np.ascontiguousarray(x[b].T).astype(cdt),
                "wqT": np.ascontiguousarray(Wq[r, :].T).astype(cdt),
                "wkT": np.ascontiguousarray(Wk[r, :].T).astype(cdt),
                "wvT": np.ascontiguousarray(Wv[r, :].T).astype(cdt),
                "woT": np.ascontiguousarray(Wo[:, r].T).astype(cdt),
                "sel": sel,
            }
        )

    res = run_bass_kernel_spmd(
        nc, in_maps, core_ids=list(range(NCORES)), trace=TRACE
    )
    if TRACE:
        LAST_EXEC_TIME_NS = res.exec_time_ns

    out = np.empty((B, S, E), dtype=np.float32)
    for b in range(B):
        out[b] = res.results[2 * b]["out"] + res.results[2 * b + 1]["out"]
    return out


# revision 20
# speedup vs baseline: 1.2417x; 1.2417x over previous
"""Multi-head attention Trainium2 kernel (8 NeuronCores, Bass/Tile).

Sharding: core c -> (batch b = c//2, head-group hg = c%2). Each core computes
attention for 8 of the 16 heads of one batch element plus its partial
out-projection; the host sums the two head-group partials per batch.

Per-core layouts (host pre-transposes inputs; contraction dims on partitions):
  xT  [E=1024, S=2048]      x[b].T
  wqT/wkT/wvT [1024, 512]   W[hg_rows].T
  woT [512, 1024]           Wo[:, hg_cols].T
  sel [8, 512]              0/1 selector for softmax-denominator replication

On-chip pipeline (all fp32):
  QT = wqT.T-tiles @ xT   [512, 2048] (head-major, transposed)
  KT likewise; V natural [2048, 512] with a ones-column appended per head
  scoresT[t,s] = KT_h.T-tile @ QT_h  (K=64, two heads row-packed per PE pass)
  expT = exp(scoresT/8) on ScalarE, batched [128, 2048] over 4 psum banks
  (outT | Z) = [V_h | 1].T @ expT    (M=65 matmul: row 64 = softmax sums)
  outT_norm = outT * replicate(1/Z)  (K=8 selector matmul + DVE mult)
  out = outT_norm.T-tiles @ woT      [2048, 1024] partial
"""

import os
import sys
import types

import numpy as np

B, S, E, H = 4, 2048, 1024, 16
DK = E // H  # 64
HG = H // 2  # heads per core = 8
DG = HG * DK  # 512 projected dims per core
NCORES = 8

TRACE = bool(os.environ.get("TRN_KERNEL_TRACE"))
# matmul-operand dtype: bf16 single-pass PE (fp32 PSUM accumulation) vs
# fp32 operands (PE double-pumps each matmul -> ~2x slower)
MM_DTYPE = os.environ.get("TRN_MM_DTYPE", "bf16")
LAST_EXEC_TIME_NS = None

_cache = {}


def _env_setup():
    import antenv

    if "antenv.axon_hooks" not in sys.modules:
        mod = types.ModuleType("antenv.axon_hooks")
        mod._hook = None
        mod.set_axon_ntff_profile_hook = lambda h: setattr(mod, "_hook", h)
        mod.get_axon_ntff_profile_hook = lambda: mod._hook
        sys.modules["antenv.axon_hooks"] = mod
        antenv.axon_hooks = mod
        try:
            from trn_agent_boot.trn_boot import _ntff_profile_via_ctypes

            mod.set_axon_ntff_profile_hook(
                _ntff_profile_via_ctypes("/opt/axon/libaxon_pjrt.so")
            )
        except Exception:
            pass

    import concourse.bass_utils as bass_utils

    bass_utils.upload_artifacts = lambda tmpdir: tmpdir

    import concourse.tile as tile
    from concourse import mybir
    from concourse.vector_clock import ScopedClock

    if getattr(tile.TileContext, "_wait_split_patched", False):
        return

    MAX_WAITS = 1  # walrus on this image rejects >1 sync wait per instruction

    def _drain_and_barrier_split(self, tick_clock, wait_clock):
        probe = self.nc.sync.drain()
        wait_clock.add_sem_waits(
            probe.ins, ScopedClock({None: tick_clock.global_clock})
        )
        waits = list(probe.ins.sync_info.on_wait)
        if len(waits) > MAX_WAITS:
            num2h = {h.num: h for h in self.sems.allocated().values()}
            probe.ins.sync_info.on_wait = []
            for w in waits:
                self.nc.sync.wait_ge(num2h[w.id], w.wait_value)
            self.nc.sync.drain()
        self.nc.all_engine_barrier()
        popped = self.nc._tile_sem_poison_stack.pop()
        assert popped is self._sem_poison
        self.nc.clear_and_free_semaphores(list(self.sems.allocated().values()))
        self.nc.all_engine_barrier()

    _orig_commit = tile.TileContext._commit_instruction
    _ctr = [0]

    def _commit_split_waits(self, inst, lazy_reg_writes=True):
        si = inst.sync_info
        if (
            si is not None
            and len(si.on_wait) > MAX_WAITS
            and inst.engine != mybir.EngineType.Unassigned
        ):
            waits = list(si.on_wait)
            keep, hoist = waits[:MAX_WAITS], waits[MAX_WAITS:]
            for i in range(0, len(hoist), MAX_WAITS):
                _ctr[0] += 1
                nop = mybir.InstNoOp(name=f"waitnop-{_ctr[0]}", ins=[], outs=[])
                nop.engine = inst.engine
                nop.sync_info = mybir.SyncInfo(
                    on_wait=hoist[i : i + MAX_WAITS], on_update=[]
                )
                self.nc.register_instruction(nop, overwrite=True)
                _orig_commit(self, nop, lazy_reg_writes=False)
            inst.sync_info = mybir.SyncInfo(on_wait=keep, on_update=list(si.on_update))
        return _orig_commit(self, inst, lazy_reg_writes=lazy_reg_writes)

    tile.TileContext._drain_and_barrier = _drain_and_barrier_split
    tile.TileContext._commit_instruction = _commit_split_waits
    tile.TileContext._wait_split_patched = True

    # use the full usable SBUF on trn2 (default constant is stale)
    import concourse.tile_utils as tile_utils

    tile_utils.max_sbuf_usage = 206 * 1024


def _build_nc():
    import contextlib

    import concourse.bass as bass
    import concourse.tile as tile
    from concourse import mybir

    F32 = mybir.dt.float32
    CDT = mybir.dt.bfloat16 if MM_DTYPE == "bf16" else mybir.dt.float32
    PS = bass.MemorySpace.PSUM
    AF = mybir.ActivationFunctionType

    nc = bass.Bass()
    xT_d = nc.dram_tensor("xT", [E, S], CDT, kind="ExternalInput")
    wqT_d = nc.dram_tensor("wqT", [E, DG], CDT, kind="ExternalInput")
    wkT_d = nc.dram_tensor("wkT", [E, DG], CDT, kind="ExternalInput")
    wvT_d = nc.dram_tensor("wvT", [E, DG], CDT, kind="ExternalInput")
    woT_d = nc.dram_tensor("woT", [DG, E], CDT, kind="ExternalInput")
    sel_d = nc.dram_tensor("sel", [HG, 512], F32, kind="ExternalInput")
    out_d = nc.dram_tensor("out", [S, E], F32, kind="ExternalOutput")

    NE = E // 128  # 8 e-tiles
    NT = S // 128  # 16 t/s-tiles
    NNC = S // 512  # 4 s-chunks
    NM = DG // 128  # 4 head-pair tiles

    with tile.TileContext(nc) as tc:
        with tc.tile_pool(name="persist", bufs=1) as pp:
            sel_sb = pp.tile([HG, 512], F32, tag="sel")
            nc.sync.dma_start(sel_sb[:], sel_d[:])

            QT = pp.tile([128, NM * S], CDT, tag="QT")  # [128, 8192]
            KT = pp.tile([128, NM * S], CDT, tag="KT")
            Vsb = pp.tile([128, NT * DG], CDT, tag="V")  # [128, 8192]
            ones = pp.tile([128, 1], CDT, tag="ones")
            nc.gpsimd.memset(ones[:], 1.0)

            # ---- projections (scoped psum + x/weight pools free afterwards) --
            with (
                tc.tile_pool(name="projps", bufs=3, space=PS) as proj_ps,
                tc.tile_pool(name="xtp", bufs=1) as xtp,
            ):
                xT = xtp.tile([128, NE * S], CDT, tag="xT")  # [128, 16384]
                for j in range(NE):
                    nc.sync.dma_start(
                        xT[:, j * S : (j + 1) * S], xT_d[j * 128 : (j + 1) * 128, :]
                    )

                for name, w_d, dst in (("q", wqT_d, QT), ("k", wkT_d, KT)):
                    with tc.tile_pool(name=f"w{name}", bufs=1) as wp:
                        wT = wp.tile([128, NE * DG], CDT, tag=f"w{name}T")
                        for j in range(NE):
                            nc.sync.dma_start(
                                wT[:, j * DG : (j + 1) * DG],
                                w_d[j * 128 : (j + 1) * 128, :],
                            )
                        for m in range(NM):
                            for n in range(NNC):
                                acc = proj_ps.tile([128, 512], F32, tag="acc")
                                for j in range(NE):
                                    nc.tensor.matmul(
                                        acc[:],
                                        wT[:, j * DG + m * 128 : j * DG + (m + 1) * 128],
                                        xT[:, j * S + n * 512 : j * S + (n + 1) * 512],
                                        start=(j == 0),
                                        stop=(j == NE - 1),
                                    )
                                nc.vector.tensor_copy(
                                    dst[:, m * S + n * 512 : m * S + (n + 1) * 512],
                                    acc[:],
                                )

                with tc.tile_pool(name="wv", bufs=1) as wp:
                    wvT = wp.tile([128, NE * DG], CDT, tag="wvT")
                    for j in range(NE):
                        nc.sync.dma_start(
                            wvT[:, j * DG : (j + 1) * DG],
                            wvT_d[j * 128 : (j + 1) * 128, :],
                        )
                    for i in range(NT):
                        acc = proj_ps.tile([128, 512], F32, tag="acc")
                        for j in range(NE):
                            nc.tensor.matmul(
                                acc[:],
                                xT[:, j * S + i * 128 : j * S + (i + 1) * 128],
                                wvT[:, j * DG : (j + 1) * DG],
                                start=(j == 0),
                                stop=(j == NE - 1),
                            )
                        nc.vector.tensor_copy(Vsb[:, i * DG : (i + 1) * DG], acc[:])

            # ---- attention phase pools (reuse freed SBUF/PSUM) ----
            attn_stack = contextlib.ExitStack()
            pp2 = attn_stack.enter_context(tc.tile_pool(name="persist2", bufs=1))
            onorm = pp2.tile([128, NM * S], CDT, tag="onorm")
            zbuf = pp2.tile([HG, S], F32, tag="zbuf")
            zrec = pp2.tile([HG, S], F32, tag="zrec")
            zstage = pp2.tile([128, NM * 512], F32, tag="zstage")
            woT = pp2.tile([128, NM * E], CDT, tag="woT")  # [128, 4096]
            for k in range(NM):
                nc.sync.dma_start(
                    woT[:, k * E : (k + 1) * E], woT_d[k * 128 : (k + 1) * 128, :]
                )
            stg = attn_stack.enter_context(tc.tile_pool(name="stage", bufs=6))
            expp = attn_stack.enter_context(tc.tile_pool(name="expp", bufs=4))
            outp = attn_stack.enter_context(tc.tile_pool(name="outp", bufs=2))
            sc_ps = attn_stack.enter_context(
                tc.tile_pool(name="scpsum", bufs=2, space=PS)
            )
            av_ps = attn_stack.enter_context(
                tc.tile_pool(name="avpsum", bufs=2, space=PS)
            )
            z_ps = attn_stack.enter_context(tc.tile_pool(name="zpsum", bufs=1, space=PS))
            ms_ps = attn_stack.enter_context(
                tc.tile_pool(name="miscpsum", bufs=1, space=PS)
            )

            def score_step(hp, n, t):
                """scores pair -> exp; returns the exp'd tile."""
                sc = sc_ps.tile([128, 1024], F32, tag="sc")
                nc.tensor.matmul(
                    sc[:, 0:512],
                    KT[0:64, hp * S + t * 128 : hp * S + (t + 1) * 128],
                    QT[0:64, hp * S + n * 512 : hp * S + (n + 1) * 512],
                )
                nc.tensor.matmul(
                    sc[:, 512:1024],
                    KT[64:128, hp * S + t * 128 : hp * S + (t + 1) * 128],
                    QT[64:128, hp * S + n * 512 : hp * S + (n + 1) * 512],
                )
                ex = expp.tile([128, 1024], CDT, tag="ex")
                nc.scalar.activation(ex[:], sc[:], AF.Exp, scale=0.125)
                return ex

            def av_step(hp, t, av, zz, zrow, ex):
                """AV pair + softmax-denominator pair consuming one exp'd tile."""
                voff = t * DG
                nc.tensor.matmul(
                    av[0:64, :],
                    Vsb[:, voff + (2 * hp) * DK : voff + (2 * hp) * DK + DK],
                    ex[:, 0:512],
                    start=(t == 0),
                    stop=(t == NT - 1),
                    tile_position=(0, 0),
                    skip_group_check=True,
                )
                nc.tensor.matmul(
                    av[64:128, :],
                    Vsb[:, voff + (2 * hp + 1) * DK : voff + (2 * hp + 1) * DK + DK],
                    ex[:, 512:1024],
                    start=(t == 0),
                    stop=(t == NT - 1),
                    tile_position=(0, 64),
                    skip_group_check=True,
                )
                nc.tensor.matmul(
                    zz[zrow : zrow + 1, :],
                    ones[:, 0:1],
                    ex[:, 0:512],
                    start=(t == 0),
                    stop=(t == NT - 1),
                    tile_position=(0, zrow),
                    skip_group_check=True,
                )
                nc.tensor.matmul(
                    zz[zrow + 32 : zrow + 33, :],
                    ones[:, 0:1],
                    ex[:, 512:1024],
                    start=(t == 0),
                    stop=(t == NT - 1),
                    tile_position=(0, zrow + 32),
                    skip_group_check=True,
                )

            for n in range(NNC):
                uos = {}
                # two interleaved head-pair streams, software-pipelined one step:
                # scores/exp(t) are emitted before av/sums(t-1) so the in-order
                # PE stream never blocks the next exp behind exp-dependent MMs
                for hpp in (0, 2):
                    av_a = av_ps.tile([128, 512], F32, tag="av")
                    av_b = av_ps.tile([128, 512], F32, tag="av")
                    zz = z_ps.tile([128, 512], F32, tag="zz")
                    pend = None
                    for t in range(NT + 1):
                        cur = None
                        if t < NT:
                            ex_a = score_step(hpp, n, t)
                            ex_b = score_step(hpp + 1, n, t)
                            cur = (ex_a, ex_b)
                        if pend is not None:
                            av_step(hpp, t - 1, av_a, zz, 0, pend[0])
                            av_step(hpp + 1, t - 1, av_b, zz, 64, pend[1])
                        pend = cur
                    for hp, av, zrow in ((hpp, av_a, 0), (hpp + 1, av_b, 64)):
                        uo = stg.tile([128, 512], F32, tag="uo")
                        nc.vector.tensor_copy(uo[:], av[:])
                        nc.vector.tensor_copy(
                            zstage[0:1, hp * 512 : (hp + 1) * 512],
                            zz[zrow : zrow + 1, :],
                        )
                        nc.vector.tensor_copy(
                            zstage[32:33, hp * 512 : (hp + 1) * 512],
                            zz[zrow + 32 : zrow + 33, :],
                        )
                        uos[hp] = uo

                # zbuf rows 0-3 = even heads of pair hp, rows 4-7 = odd heads
                for hp in range(NM):
                    nc.sync.dma_start(
                        zbuf[hp : hp + 1, n * 512 : (n + 1) * 512],
                        zstage[0:1, hp * 512 : (hp + 1) * 512],
                    )
                    nc.sync.dma_start(
                        zbuf[4 + hp : 5 + hp, n * 512 : (n + 1) * 512],
                        zstage[32:33, hp * 512 : (hp + 1) * 512],
                    )
                nc.vector.reciprocal(
                    zrec[:, n * 512 : (n + 1) * 512], zbuf[:, n * 512 : (n + 1) * 512]
                )
                for k in range(NM):
                    rep = ms_ps.tile([128, 512], F32, tag="ms")
                    nc.tensor.matmul(
                        rep[:],
                        sel_sb[:, k * 128 : (k + 1) * 128],
                        zrec[:, n * 512 : (n + 1) * 512],
                    )
                    nc.vector.tensor_tensor(
                        onorm[:, k * S + n * 512 : k * S + (n + 1) * 512],
                        uos[k][:],
                        rep[:],
                        mybir.AluOpType.mult,
                    )

                for i in range(4 * n, 4 * n + 4):
                    osb = outp.tile([128, E], F32, tag="osb")
                    for eh in range(2):
                        ps = ms_ps.tile([128, 512], F32, tag="ms")
                        for k in range(NM):
                            nc.tensor.matmul(
                                ps[:],
                                onorm[:, k * S + i * 128 : k * S + (i + 1) * 128],
                                woT[:, k * E + eh * 512 : k * E + (eh + 1) * 512],
                                start=(k == 0),
                                stop=(k == NM - 1),
                            )
                        nc.vector.tensor_copy(osb[:, eh * 512 : (eh + 1) * 512], ps[:])
                    nc.sync.dma_start(out_d[i * 128 : (i + 1) * 128, :], osb[:])

            attn_stack.close()

    return nc


def _make_sel():
    # zbuf row for head (2k + p//64): even heads -> row k, odd heads -> row 4+k
    sel = np.zeros((HG, 512), dtype=np.float32)
    for k in range(4):
        for p in range(128):
            r = k if p < 64 else 4 + k
            sel[r, k * 128 + p] = 1.0
    return sel


def kernel(x, Wq, Wk, Wv, Wo):
    global LAST_EXEC_TIME_NS
    _env_setup()
    from concourse.bass_utils import run_bass_kernel_spmd

    x = np.asarray(x, dtype=np.float32)
    Wq = np.asarray(Wq, dtype=np.float32)
    Wk = np.asarray(Wk, dtype=np.float32)
    Wv = np.asarray(Wv, dtype=np.float32)
    Wo = np.asarray(Wo, dtype=np.float32)

    if "nc" not in _cache:
        _cache["nc"] = _build_nc()
    nc = _cache["nc"]

    if MM_DTYPE == "bf16":
        import ml_dtypes

        cdt = ml_dtypes.bfloat16
    else:
        cdt = np.float32

    sel = _make_sel()
    in_maps = []
    for c in range(NCORES):
        b, hg = c // 2, c % 2
        r = slice(hg * DG, (hg + 1) * DG)
        in_maps.append(
            {
                "xT": np.ascontiguousarray(x[b].T).astype(cdt),
                "wqT": np.ascontiguousarray(Wq[r, :].T).astype(cdt),
                "wkT": np.ascontiguousarray(Wk[r, :].T).astype(cdt),
                "wvT": np.ascontiguousarray(Wv[r, :].T).astype(cdt),
                "woT": np.ascontiguousarray(Wo[:, r].T).astype(cdt),
                "sel": sel,
            }
        )

    res = run_bass_kernel_spmd(
        nc, in_maps, core_ids=list(range(NCORES)), trace=TRACE
    )
    if TRACE:
        LAST_EXEC_TIME_NS = res.exec_time_ns

    out = np.empty((B, S, E), dtype=np.float32)
    for b in range(B):
        out[b] = res.results[2 * b]["out"] + res.results[2 * b + 1]["out"]
    return out


# revision 21
# speedup vs baseline: 1.3024x; 1.0489x over previous
"""Multi-head attention Trainium2 kernel (8 NeuronCores, Bass/Tile).

Sharding: core c -> (batch b = c//2, head-group hg = c%2). Each core computes
attention for 8 of the 16 heads of one batch element plus its partial
out-projection; the host sums the two head-group partials per batch.

Per-core layouts (host pre-transposes inputs; contraction dims on partitions):
  xT  [E=1024, S=2048]      x[b].T
  wqT/wkT/wvT [1024, 512]   W[hg_rows].T
  woT [512, 1024]           Wo[:, hg_cols].T
  sel [8, 512]              0/1 selector for softmax-denominator replication

On-chip pipeline (all fp32):
  QT = wqT.T-tiles @ xT   [512, 2048] (head-major, transposed)
  KT likewise; V natural [2048, 512] with a ones-column appended per head
  scoresT[t,s] = KT_h.T-tile @ QT_h  (K=64, two heads row-packed per PE pass)
  expT = exp(scoresT/8) on ScalarE, batched [128, 2048] over 4 psum banks
  (outT | Z) = [V_h | 1].T @ expT    (M=65 matmul: row 64 = softmax sums)
  outT_norm = outT * replicate(1/Z)  (K=8 selector matmul + DVE mult)
  out = outT_norm.T-tiles @ woT      [2048, 1024] partial
"""

import os
import sys
import types

import numpy as np

B, S, E, H = 4, 2048, 1024, 16
DK = E // H  # 64
HG = H // 2  # heads per core = 8
DG = HG * DK  # 512 projected dims per core
NCORES = 8

TRACE = bool(os.environ.get("TRN_KERNEL_TRACE"))
# matmul-operand dtype: bf16 single-pass PE (fp32 PSUM accumulation) vs
# fp32 operands (PE double-pumps each matmul -> ~2x slower)
MM_DTYPE = os.environ.get("TRN_MM_DTYPE", "bf16")
LAST_EXEC_TIME_NS = None

_cache = {}


def _env_setup():
    import antenv

    if "antenv.axon_hooks" not in sys.modules:
        mod = types.ModuleType("antenv.axon_hooks")
        mod._hook = None
        mod.set_axon_ntff_profile_hook = lambda h: setattr(mod, "_hook", h)
        mod.get_axon_ntff_profile_hook = lambda: mod._hook
        sys.modules["antenv.axon_hooks"] = mod
        antenv.axon_hooks = mod
        try:
            from trn_agent_boot.trn_boot import _ntff_profile_via_ctypes

            mod.set_axon_ntff_profile_hook(
                _ntff_profile_via_ctypes("/opt/axon/libaxon_pjrt.so")
            )
        except Exception:
            pass

    import concourse.bass_utils as bass_utils

    bass_utils.upload_artifacts = lambda tmpdir: tmpdir

    import concourse.tile as tile
    from concourse import mybir
    from concourse.vector_clock import ScopedClock

    if getattr(tile.TileContext, "_wait_split_patched", False):
        return

    MAX_WAITS = 1  # walrus on this image rejects >1 sync wait per instruction

    def _drain_and_barrier_split(self, tick_clock, wait_clock):
        probe = self.nc.sync.drain()
        wait_clock.add_sem_waits(
            probe.ins, ScopedClock({None: tick_clock.global_clock})
        )
        waits = list(probe.ins.sync_info.on_wait)
        if len(waits) > MAX_WAITS:
            num2h = {h.num: h for h in self.sems.allocated().values()}
            probe.ins.sync_info.on_wait = []
            for w in waits:
                self.nc.sync.wait_ge(num2h[w.id], w.wait_value)
            self.nc.sync.drain()
        self.nc.all_engine_barrier()
        popped = self.nc._tile_sem_poison_stack.pop()
        assert popped is self._sem_poison
        self.nc.clear_and_free_semaphores(list(self.sems.allocated().values()))
        self.nc.all_engine_barrier()

    _orig_commit = tile.TileContext._commit_instruction
    _ctr = [0]

    def _commit_split_waits(self, inst, lazy_reg_writes=True):
        si = inst.sync_info
        if (
            si is not None
            and len(si.on_wait) > MAX_WAITS
            and inst.engine != mybir.EngineType.Unassigned
        ):
            waits = list(si.on_wait)
            keep, hoist = waits[:MAX_WAITS], waits[MAX_WAITS:]
            for i in range(0, len(hoist), MAX_WAITS):
                _ctr[0] += 1
                nop = mybir.InstNoOp(name=f"waitnop-{_ctr[0]}", ins=[], outs=[])
                nop.engine = inst.engine
                nop.sync_info = mybir.SyncInfo(
                    on_wait=hoist[i : i + MAX_WAITS], on_update=[]
                )
                self.nc.register_instruction(nop, overwrite=True)
                _orig_commit(self, nop, lazy_reg_writes=False)
            inst.sync_info = mybir.SyncInfo(on_wait=keep, on_update=list(si.on_update))
        return _orig_commit(self, inst, lazy_reg_writes=lazy_reg_writes)

    tile.TileContext._drain_and_barrier = _drain_and_barrier_split
    tile.TileContext._commit_instruction = _commit_split_waits
    tile.TileContext._wait_split_patched = True

    # use the full usable SBUF on trn2 (default constant is stale)
    import concourse.tile_utils as tile_utils

    tile_utils.max_sbuf_usage = 206 * 1024


def _build_nc():
    import contextlib

    import concourse.bass as bass
    import concourse.tile as tile
    from concourse import mybir

    F32 = mybir.dt.float32
    CDT = mybir.dt.bfloat16 if MM_DTYPE == "bf16" else mybir.dt.float32
    PS = bass.MemorySpace.PSUM
    AF = mybir.ActivationFunctionType

    nc = bass.Bass()
    xT_d = nc.dram_tensor("xT", [E, S], CDT, kind="ExternalInput")
    wqT_d = nc.dram_tensor("wqT", [E, DG], CDT, kind="ExternalInput")
    wkT_d = nc.dram_tensor("wkT", [E, DG], CDT, kind="ExternalInput")
    wvT_d = nc.dram_tensor("wvT", [E, DG], CDT, kind="ExternalInput")
    woT_d = nc.dram_tensor("woT", [DG, E], CDT, kind="ExternalInput")
    sel_d = nc.dram_tensor("sel", [HG, 512], F32, kind="ExternalInput")
    out_d = nc.dram_tensor("out", [S, E], F32, kind="ExternalOutput")

    NE = E // 128  # 8 e-tiles
    NT = S // 128  # 16 t/s-tiles
    NNC = S // 512  # 4 s-chunks
    NM = DG // 128  # 4 head-pair tiles

    with tile.TileContext(nc) as tc:
        st = contextlib.ExitStack()
        with st:
            pp = st.enter_context(tc.tile_pool(name="persist", bufs=1))
            stg = st.enter_context(tc.tile_pool(name="stage", bufs=6))
            expp = st.enter_context(tc.tile_pool(name="expp", bufs=4))
            outp = st.enter_context(tc.tile_pool(name="outp", bufs=2))
            sc_ps = st.enter_context(tc.tile_pool(name="scpsum", bufs=2, space=PS))
            av_ps = st.enter_context(tc.tile_pool(name="avpsum", bufs=2, space=PS))
            z_ps = st.enter_context(tc.tile_pool(name="zpsum", bufs=1, space=PS))
            ms_ps = st.enter_context(tc.tile_pool(name="miscpsum", bufs=1, space=PS))

            sel_sb = pp.tile([HG, 512], F32, tag="sel")
            nc.sync.dma_start(sel_sb[:], sel_d[:])

            QT = pp.tile([128, NM * S], CDT, tag="QT")  # [128, 8192]
            KT = pp.tile([128, NM * S], CDT, tag="KT")
            Vsb = pp.tile([128, NT * DG], CDT, tag="V")  # [128, 8192]
            onorm = pp.tile([128, NM * S], CDT, tag="onorm")
            zbuf = pp.tile([HG, S], F32, tag="zbuf")
            zrec = pp.tile([HG, S], F32, tag="zrec")
            zstage = pp.tile([128, NM * 512], F32, tag="zstage")
            woT = pp.tile([128, NM * E], CDT, tag="woT")  # [128, 4096]
            ones = pp.tile([128, 1], CDT, tag="ones")
            nc.gpsimd.memset(ones[:], 1.0)

            xT = pp.tile([128, NE * S], CDT, tag="xT")  # [128, 16384]
            wq = pp.tile([128, NE * DG], CDT, tag="wq")
            wk = pp.tile([128, NE * DG], CDT, tag="wk")
            wv = pp.tile([128, NE * DG], CDT, tag="wv")
            for j in range(NE):
                nc.sync.dma_start(
                    xT[:, j * S : (j + 1) * S], xT_d[j * 128 : (j + 1) * 128, :]
                )
            for w_sb, w_d in ((wq, wqT_d), (wk, wkT_d), (wv, wvT_d)):
                for j in range(NE):
                    nc.sync.dma_start(
                        w_sb[:, j * DG : (j + 1) * DG],
                        w_d[j * 128 : (j + 1) * 128, :],
                    )
            for k in range(NM):
                nc.sync.dma_start(
                    woT[:, k * E : (k + 1) * E], woT_d[k * 128 : (k + 1) * 128, :]
                )

            def qk_proj(w_sb, dst, m, n):
                acc = ms_ps.tile([128, 512], F32, tag="ms")
                for j in range(NE):
                    nc.tensor.matmul(
                        acc[:],
                        w_sb[:, j * DG + m * 128 : j * DG + (m + 1) * 128],
                        xT[:, j * S + n * 512 : j * S + (n + 1) * 512],
                        start=(j == 0),
                        stop=(j == NE - 1),
                    )
                nc.vector.tensor_copy(
                    dst[:, m * S + n * 512 : m * S + (n + 1) * 512], acc[:]
                )

            def v_proj(i):
                acc = ms_ps.tile([128, 512], F32, tag="ms")
                for j in range(NE):
                    nc.tensor.matmul(
                        acc[:],
                        xT[:, j * S + i * 128 : j * S + (i + 1) * 128],
                        wv[:, j * DG : (j + 1) * DG],
                        start=(j == 0),
                        stop=(j == NE - 1),
                    )
                nc.vector.tensor_copy(Vsb[:, i * DG : (i + 1) * DG], acc[:])

            # head-pairs 0/1 + all of V emitted up-front; the rest drips into
            # the first attention chunk as PE filler while ScalarE runs exp
            for m in (0, 1):
                for n in range(NNC):
                    qk_proj(wq, QT, m, n)
                    qk_proj(wk, KT, m, n)
            for i in range(NT):
                v_proj(i)

            filler = []
            for m in (2, 3):
                for n in range(NNC):
                    filler.append(lambda m=m, n=n: qk_proj(wq, QT, m, n))
                    filler.append(lambda m=m, n=n: qk_proj(wk, KT, m, n))

            def drip():
                if filler:
                    filler.pop(0)()

            def score_step(hp, n, t):
                sc = sc_ps.tile([128, 1024], F32, tag="sc")
                nc.tensor.matmul(
                    sc[:, 0:512],
                    KT[0:64, hp * S + t * 128 : hp * S + (t + 1) * 128],
                    QT[0:64, hp * S + n * 512 : hp * S + (n + 1) * 512],
                )
                nc.tensor.matmul(
                    sc[:, 512:1024],
                    KT[64:128, hp * S + t * 128 : hp * S + (t + 1) * 128],
                    QT[64:128, hp * S + n * 512 : hp * S + (n + 1) * 512],
                )
                ex = expp.tile([128, 1024], CDT, tag="ex")
                nc.scalar.activation(ex[:], sc[:], AF.Exp, scale=0.125)
                return ex

            def av_pair(hp, t, av, ex):
                voff = t * DG
                nc.tensor.matmul(
                    av[0:64, :],
                    Vsb[:, voff + (2 * hp) * DK : voff + (2 * hp) * DK + DK],
                    ex[:, 0:512],
                    start=(t == 0),
                    stop=(t == NT - 1),
                    tile_position=(0, 0),
                    skip_group_check=True,
                )
                nc.tensor.matmul(
                    av[64:128, :],
                    Vsb[:, voff + (2 * hp + 1) * DK : voff + (2 * hp + 1) * DK + DK],
                    ex[:, 512:1024],
                    start=(t == 0),
                    stop=(t == NT - 1),
                    tile_position=(0, 64),
                    skip_group_check=True,
                )

            def sum_pair(t, zz, zrow, ex):
                nc.tensor.matmul(
                    zz[zrow : zrow + 1, :],
                    ones[:, 0:1],
                    ex[:, 0:512],
                    start=(t == 0),
                    stop=(t == NT - 1),
                    tile_position=(0, zrow),
                    skip_group_check=True,
                )
                nc.tensor.matmul(
                    zz[zrow + 32 : zrow + 33, :],
                    ones[:, 0:1],
                    ex[:, 512:1024],
                    start=(t == 0),
                    stop=(t == NT - 1),
                    tile_position=(0, zrow + 32),
                    skip_group_check=True,
                )

            def boundary(n):
                """post-chunk-n work, dripped into chunk n+1: normalize+outproj."""
                tasks = []
                for k in range(NM):
                    def norm_k(k=k, n=n):
                        rep = ms_ps.tile([128, 512], F32, tag="ms")
                        nc.tensor.matmul(
                            rep[:],
                            sel_sb[:, k * 128 : (k + 1) * 128],
                            zrec[:, n * 512 : (n + 1) * 512],
                        )
                        nc.vector.tensor_tensor(
                            onorm[:, k * S + n * 512 : k * S + (n + 1) * 512],
                            uos[(n, k)][:],
                            rep[:],
                            mybir.AluOpType.mult,
                        )
                    tasks.append(norm_k)
                for i in range(4 * n, 4 * n + 4):
                    def oproj_i(i=i):
                        osb = outp.tile([128, E], F32, tag="osb")
                        for eh in range(2):
                            ps = ms_ps.tile([128, 512], F32, tag="ms")
                            for k in range(NM):
                                nc.tensor.matmul(
                                    ps[:],
                                    onorm[:, k * S + i * 128 : k * S + (i + 1) * 128],
                                    woT[:, k * E + eh * 512 : k * E + (eh + 1) * 512],
                                    start=(k == 0),
                                    stop=(k == NM - 1),
                                )
                            nc.vector.tensor_copy(
                                osb[:, eh * 512 : (eh + 1) * 512], ps[:]
                            )
                        nc.sync.dma_start(out_d[i * 128 : (i + 1) * 128, :], osb[:])
                    tasks.append(oproj_i)
                return tasks

            uos = {}
            for n in range(NNC):
                for hpp in (0, 2):
                    av_a = av_ps.tile([128, 512], F32, tag="av")
                    av_b = av_ps.tile([128, 512], F32, tag="av")
                    zz = z_ps.tile([128, 512], F32, tag="zz")
                    pend = None
                    for t in range(NT + 1):
                        cur = None
                        if t < NT:
                            ex_a = score_step(hpp, n, t)
                            ex_b = score_step(hpp + 1, n, t)
                            cur = (ex_a, ex_b)
                        if pend is not None:
                            av_pair(hpp, t - 1, av_a, pend[0])
                            av_pair(hpp + 1, t - 1, av_b, pend[1])
                            sum_pair(t - 1, zz, 0, pend[0])
                            sum_pair(t - 1, zz, 64, pend[1])
                        pend = cur
                        drip()
                    for hp, av, zrow in ((hpp, av_a, 0), (hpp + 1, av_b, 64)):
                        uo = stg.tile([128, 512], F32, tag="uo")
                        nc.vector.tensor_copy(uo[:], av[:])
                        nc.vector.tensor_copy(
                            zstage[0:1, hp * 512 : (hp + 1) * 512],
                            zz[zrow : zrow + 1, :],
                        )
                        nc.vector.tensor_copy(
                            zstage[32:33, hp * 512 : (hp + 1) * 512],
                            zz[zrow + 32 : zrow + 33, :],
                        )
                        uos[(n, hp)] = uo

                # zbuf rows 0-3 = even heads of pair hp, rows 4-7 = odd heads
                for hp in range(NM):
                    nc.sync.dma_start(
                        zbuf[hp : hp + 1, n * 512 : (n + 1) * 512],
                        zstage[0:1, hp * 512 : (hp + 1) * 512],
                    )
                    nc.sync.dma_start(
                        zbuf[4 + hp : 5 + hp, n * 512 : (n + 1) * 512],
                        zstage[32:33, hp * 512 : (hp + 1) * 512],
                    )
                nc.vector.reciprocal(
                    zrec[:, n * 512 : (n + 1) * 512], zbuf[:, n * 512 : (n + 1) * 512]
                )
                filler.extend(boundary(n))
            while filler:
                filler.pop(0)()

    return nc


def _make_sel():
    # zbuf row for head (2k + p//64): even heads -> row k, odd heads -> row 4+k
    sel = np.zeros((HG, 512), dtype=np.float32)
    for k in range(4):
        for p in range(128):
            r = k if p < 64 else 4 + k
            sel[r, k * 128 + p] = 1.0
    return sel


def kernel(x, Wq, Wk, Wv, Wo):
    global LAST_EXEC_TIME_NS
    _env_setup()
    from concourse.bass_utils import run_bass_kernel_spmd

    x = np.asarray(x, dtype=np.float32)
    Wq = np.asarray(Wq, dtype=np.float32)
    Wk = np.asarray(Wk, dtype=np.float32)
    Wv = np.asarray(Wv, dtype=np.float32)
    Wo = np.asarray(Wo, dtype=np.float32)

    if "nc" not in _cache:
        _cache["nc"] = _build_nc()
    nc = _cache["nc"]

    if MM_DTYPE == "bf16":
        import ml_dtypes

        cdt = ml_dtypes.bfloat16
    else:
        cdt = np.float32

    sel = _make_sel()
    in_maps = []
    for c in range(NCORES):
        b, hg = c // 2, c % 2
        r = slice(hg * DG, (hg + 1) * DG)
        in_maps.append(
            {
                "xT": np.ascontiguousarray(x[b].T).astype(cdt),
                "wqT": np.ascontiguousarray(Wq[r, :].T).astype(cdt),
                "wkT": np.ascontiguousarray(Wk[r, :].T).astype(cdt),
                "wvT": np.ascontiguousarray(Wv[r, :].T).astype(cdt),
                "woT": np.ascontiguousarray(Wo[:, r].T).astype(cdt),
                "sel": sel,
            }
        )

    res = run_bass_kernel_spmd(
        nc, in_maps, core_ids=list(range(NCORES)), trace=TRACE
    )
    if TRACE:
        LAST_EXEC_TIME_NS = res.exec_time_ns

    out = np.empty((B, S, E), dtype=np.float32)
    for b in range(B):
        out[b] = res.results[2 * b]["out"] + res.results[2 * b + 1]["out"]
    return out


# revision 24
# speedup vs baseline: 1.3433x; 1.0315x over previous
"""Multi-head attention Trainium2 kernel (8 NeuronCores, Bass/Tile).

Sharding: core c -> (batch b = c//2, head-group hg = c%2). Each core computes
attention for 8 of the 16 heads of one batch element plus its partial
out-projection; the host sums the two head-group partials per batch.

Per-core layouts (host pre-transposes inputs; contraction dims on partitions):
  xT  [E=1024, S=2048]      x[b].T
  wqT/wkT/wvT [1024, 512]   W[hg_rows].T
  woT [512, 1024]           Wo[:, hg_cols].T
  sel [8, 512]              0/1 selector for softmax-denominator replication

On-chip pipeline (all fp32):
  QT = wqT.T-tiles @ xT   [512, 2048] (head-major, transposed)
  KT likewise; V natural [2048, 512] with a ones-column appended per head
  scoresT[t,s] = KT_h.T-tile @ QT_h  (K=64, two heads row-packed per PE pass)
  expT = exp(scoresT/8) on ScalarE, batched [128, 2048] over 4 psum banks
  (outT | Z) = [V_h | 1].T @ expT    (M=65 matmul: row 64 = softmax sums)
  outT_norm = outT * replicate(1/Z)  (K=8 selector matmul + DVE mult)
  out = outT_norm.T-tiles @ woT      [2048, 1024] partial
"""

import os
import sys
import types

import numpy as np

B, S, E, H = 4, 2048, 1024, 16
DK = E // H  # 64
HG = H // 2  # heads per core = 8
DG = HG * DK  # 512 projected dims per core
NCORES = 8

TRACE = bool(os.environ.get("TRN_KERNEL_TRACE"))
# matmul-operand dtype: bf16 single-pass PE (fp32 PSUM accumulation) vs
# fp32 operands (PE double-pumps each matmul -> ~2x slower)
MM_DTYPE = os.environ.get("TRN_MM_DTYPE", "bf16")
LAST_EXEC_TIME_NS = None

_cache = {}


def _env_setup():
    import antenv

    if "antenv.axon_hooks" not in sys.modules:
        mod = types.ModuleType("antenv.axon_hooks")
        mod._hook = None
        mod.set_axon_ntff_profile_hook = lambda h: setattr(mod, "_hook", h)
        mod.get_axon_ntff_profile_hook = lambda: mod._hook
        sys.modules["antenv.axon_hooks"] = mod
        antenv.axon_hooks = mod
        try:
            from trn_agent_boot.trn_boot import _ntff_profile_via_ctypes

            mod.set_axon_ntff_profile_hook(
                _ntff_profile_via_ctypes("/opt/axon/libaxon_pjrt.so")
            )
        except Exception:
            pass

    import concourse.bass_utils as bass_utils

    bass_utils.upload_artifacts = lambda tmpdir: tmpdir

    import concourse.tile as tile
    from concourse import mybir
    from concourse.vector_clock import ScopedClock

    if getattr(tile.TileContext, "_wait_split_patched", False):
        return

    MAX_WAITS = 1  # walrus on this image rejects >1 sync wait per instruction

    def _drain_and_barrier_split(self, tick_clock, wait_clock):
        probe = self.nc.sync.drain()
        wait_clock.add_sem_waits(
            probe.ins, ScopedClock({None: tick_clock.global_clock})
        )
        waits = list(probe.ins.sync_info.on_wait)
        if len(waits) > MAX_WAITS:
            num2h = {h.num: h for h in self.sems.allocated().values()}
            probe.ins.sync_info.on_wait = []
            for w in waits:
                self.nc.sync.wait_ge(num2h[w.id], w.wait_value)
            self.nc.sync.drain()
        self.nc.all_engine_barrier()
        popped = self.nc._tile_sem_poison_stack.pop()
        assert popped is self._sem_poison
        self.nc.clear_and_free_semaphores(list(self.sems.allocated().values()))
        self.nc.all_engine_barrier()

    _orig_commit = tile.TileContext._commit_instruction
    _ctr = [0]

    def _commit_split_waits(self, inst, lazy_reg_writes=True):
        si = inst.sync_info
        if (
            si is not None
            and len(si.on_wait) > MAX_WAITS
            and inst.engine != mybir.EngineType.Unassigned
        ):
            waits = list(si.on_wait)
            keep, hoist = waits[:MAX_WAITS], waits[MAX_WAITS:]
            for i in range(0, len(hoist), MAX_WAITS):
                _ctr[0] += 1
                nop = mybir.InstNoOp(name=f"waitnop-{_ctr[0]}", ins=[], outs=[])
                nop.engine = inst.engine
                nop.sync_info = mybir.SyncInfo(
                    on_wait=hoist[i : i + MAX_WAITS], on_update=[]
                )
                self.nc.register_instruction(nop, overwrite=True)
                _orig_commit(self, nop, lazy_reg_writes=False)
            inst.sync_info = mybir.SyncInfo(on_wait=keep, on_update=list(si.on_update))
        return _orig_commit(self, inst, lazy_reg_writes=lazy_reg_writes)

    tile.TileContext._drain_and_barrier = _drain_and_barrier_split
    tile.TileContext._commit_instruction = _commit_split_waits
    tile.TileContext._wait_split_patched = True

    # use the full usable SBUF on trn2 (default constant is stale)
    import concourse.tile_utils as tile_utils

    tile_utils.max_sbuf_usage = 206 * 1024

    if os.environ.get("TRN_LDW_OPT"):
        _orig_bvo = bass_utils.bir_verify_and_optimise

        def _bvo_ldwopt(*a, **kw):
            orig_run = bass_utils.run_command

            def run_patched(cmd, **rkw):
                cmd = [
                    c.replace("--enable-ldw-opt=false", "--enable-ldw-opt=true")
                    if isinstance(c, str)
                    else c
                    for c in cmd
                ]
                return orig_run(cmd, **rkw)

            bass_utils.run_command = run_patched
            try:
                return _orig_bvo(*a, **kw)
            finally:
                bass_utils.run_command = orig_run

        bass_utils.bir_verify_and_optimise = _bvo_ldwopt


def _build_nc():
    import contextlib

    import concourse.bass as bass
    import concourse.tile as tile
    from concourse import mybir

    F32 = mybir.dt.float32
    CDT = mybir.dt.bfloat16 if MM_DTYPE == "bf16" else mybir.dt.float32
    PS = bass.MemorySpace.PSUM
    AF = mybir.ActivationFunctionType

    nc = bass.Bass()
    xT_d = nc.dram_tensor("xT", [E, S], CDT, kind="ExternalInput")
    wqT_d = nc.dram_tensor("wqT", [E, DG], CDT, kind="ExternalInput")
    wkT_d = nc.dram_tensor("wkT", [E, DG], CDT, kind="ExternalInput")
    wvT_d = nc.dram_tensor("wvT", [E, DG], CDT, kind="ExternalInput")
    woT_d = nc.dram_tensor("woT", [DG, E], CDT, kind="ExternalInput")
    sel_d = nc.dram_tensor("sel", [HG, 512], CDT, kind="ExternalInput")
    out_d = nc.dram_tensor("out", [S, E], F32, kind="ExternalOutput")

    NE = E // 128  # 8 e-tiles
    NT = S // 128  # 16 t/s-tiles
    NNC = S // 512  # 4 s-chunks
    NM = DG // 128  # 4 head-pair tiles

    with tile.TileContext(nc) as tc:
        st = contextlib.ExitStack()
        with st:
            pp = st.enter_context(tc.tile_pool(name="persist", bufs=1))
            stg = st.enter_context(tc.tile_pool(name="stage", bufs=6))
            expp = st.enter_context(tc.tile_pool(name="expp", bufs=4))
            outp = st.enter_context(tc.tile_pool(name="outp", bufs=2))
            sc_ps = st.enter_context(tc.tile_pool(name="scpsum", bufs=2, space=PS))
            av_ps = st.enter_context(tc.tile_pool(name="avpsum", bufs=2, space=PS))
            z_ps = st.enter_context(tc.tile_pool(name="zpsum", bufs=1, space=PS))
            ms_ps = st.enter_context(tc.tile_pool(name="miscpsum", bufs=1, space=PS))

            sel_sb = pp.tile([HG, 512], CDT, tag="sel")
            nc.sync.dma_start(sel_sb[:], sel_d[:])

            QT = pp.tile([128, NM * S], CDT, tag="QT")  # [128, 8192]
            KT = pp.tile([128, NM * S], CDT, tag="KT")
            Vsb = pp.tile([128, NT * DG], CDT, tag="V")  # [128, 8192]
            onorm = pp.tile([128, NM * S], CDT, tag="onorm")
            zbuf = pp.tile([HG, S], F32, tag="zbuf")
            zrec = pp.tile([HG, S], F32, tag="zrec")
            zrecc = pp.tile([HG, S], CDT, tag="zrecc")
            zstage = pp.tile([128, NM * 512], F32, tag="zstage")
            woT = pp.tile([128, NM * E], CDT, tag="woT")  # [128, 4096]
            ones = pp.tile([128, 1], CDT, tag="ones")
            nc.gpsimd.memset(ones[:], 1.0)

            xT = pp.tile([128, NE * S], CDT, tag="xT")  # [128, 16384]
            wq = pp.tile([128, NE * DG], CDT, tag="wq")
            wk = pp.tile([128, NE * DG], CDT, tag="wk")
            wv = pp.tile([128, NE * DG], CDT, tag="wv")
            for j in range(NE):
                nc.sync.dma_start(
                    xT[:, j * S : (j + 1) * S], xT_d[j * 128 : (j + 1) * 128, :]
                )
            for w_sb, w_d in ((wq, wqT_d), (wk, wkT_d), (wv, wvT_d)):
                for j in range(NE):
                    nc.sync.dma_start(
                        w_sb[:, j * DG : (j + 1) * DG],
                        w_d[j * 128 : (j + 1) * 128, :],
                    )
            for k in range(NM):
                nc.sync.dma_start(
                    woT[:, k * E : (k + 1) * E], woT_d[k * 128 : (k + 1) * 128, :]
                )

            def qk_proj(w_sb, dst, m, n):
                acc = ms_ps.tile([128, 512], F32, tag="ms")
                for j in range(NE):
                    nc.tensor.matmul(
                        acc[:],
                        w_sb[:, j * DG + m * 128 : j * DG + (m + 1) * 128],
                        xT[:, j * S + n * 512 : j * S + (n + 1) * 512],
                        start=(j == 0),
                        stop=(j == NE - 1),
                    )
                nc.vector.tensor_copy(
                    dst[:, m * S + n * 512 : m * S + (n + 1) * 512], acc[:]
                )

            def v_proj(i):
                acc = ms_ps.tile([128, 512], F32, tag="ms")
                for j in range(NE):
                    nc.tensor.matmul(
                        acc[:],
                        xT[:, j * S + i * 128 : j * S + (i + 1) * 128],
                        wv[:, j * DG : (j + 1) * DG],
                        start=(j == 0),
                        stop=(j == NE - 1),
                    )
                nc.vector.tensor_copy(Vsb[:, i * DG : (i + 1) * DG], acc[:])

            # head-pairs 0/1 + all of V emitted up-front; the rest drips into
            # the first attention chunk as PE filler while ScalarE runs exp
            for m in (0, 1):
                for n in range(NNC):
                    qk_proj(wq, QT, m, n)
                    qk_proj(wk, KT, m, n)
            for i in range(NT):
                v_proj(i)

            filler = []
            for m in (2, 3):
                for n in range(NNC):
                    filler.append(lambda m=m, n=n: qk_proj(wq, QT, m, n))
                    filler.append(lambda m=m, n=n: qk_proj(wk, KT, m, n))

            def drip():
                if filler:
                    filler.pop(0)()

            def score_step(hp, n, t):
                sc = sc_ps.tile([128, 1024], F32, tag="sc")
                nc.tensor.matmul(
                    sc[:, 0:512],
                    KT[0:64, hp * S + t * 128 : hp * S + (t + 1) * 128],
                    QT[0:64, hp * S + n * 512 : hp * S + (n + 1) * 512],
                )
                nc.tensor.matmul(
                    sc[:, 512:1024],
                    KT[64:128, hp * S + t * 128 : hp * S + (t + 1) * 128],
                    QT[64:128, hp * S + n * 512 : hp * S + (n + 1) * 512],
                )
                ex = expp.tile([128, 1024], CDT, tag="ex")
                nc.scalar.activation(ex[:], sc[:], AF.Exp, scale=0.125)
                return ex

            def av_pair(hp, t, av, ex):
                voff = t * DG
                nc.tensor.matmul(
                    av[0:64, :],
                    Vsb[:, voff + (2 * hp) * DK : voff + (2 * hp) * DK + DK],
                    ex[:, 0:512],
                    start=(t == 0),
                    stop=(t == NT - 1),
                    tile_position=(0, 0),
                    skip_group_check=True,
                )
                nc.tensor.matmul(
                    av[64:128, :],
                    Vsb[:, voff + (2 * hp + 1) * DK : voff + (2 * hp + 1) * DK + DK],
                    ex[:, 512:1024],
                    start=(t == 0),
                    stop=(t == NT - 1),
                    tile_position=(0, 64),
                    skip_group_check=True,
                )

            def sum_pair(t, zz, zrow, ex):
                nc.tensor.matmul(
                    zz[zrow : zrow + 1, :],
                    ones[:, 0:1],
                    ex[:, 0:512],
                    start=(t == 0),
                    stop=(t == NT - 1),
                    tile_position=(0, zrow),
                    skip_group_check=True,
                )
                nc.tensor.matmul(
                    zz[zrow + 32 : zrow + 33, :],
                    ones[:, 0:1],
                    ex[:, 512:1024],
                    start=(t == 0),
                    stop=(t == NT - 1),
                    tile_position=(0, zrow + 32),
                    skip_group_check=True,
                )

            def boundary(n):
                """post-chunk-n work, dripped into chunk n+1: normalize+outproj."""
                tasks = []
                for k in range(NM):
                    def norm_k(k=k, n=n):
                        rep = ms_ps.tile([128, 512], F32, tag="ms")
                        nc.tensor.matmul(
                            rep[:],
                            sel_sb[:, k * 128 : (k + 1) * 128],
                            zrecc[:, n * 512 : (n + 1) * 512],
                        )
                        nc.vector.tensor_tensor(
                            onorm[:, k * S + n * 512 : k * S + (n + 1) * 512],
                            uos[(n, k)][:],
                            rep[:],
                            mybir.AluOpType.mult,
                        )
                    tasks.append(norm_k)
                for i in range(4 * n, 4 * n + 4):
                    def oproj_i(i=i):
                        osb = outp.tile([128, E], F32, tag="osb")
                        for eh in range(2):
                            ps = ms_ps.tile([128, 512], F32, tag="ms")
                            for k in range(NM):
                                nc.tensor.matmul(
                                    ps[:],
                                    onorm[:, k * S + i * 128 : k * S + (i + 1) * 128],
                                    woT[:, k * E + eh * 512 : k * E + (eh + 1) * 512],
                                    start=(k == 0),
                                    stop=(k == NM - 1),
                                )
                            nc.vector.tensor_copy(
                                osb[:, eh * 512 : (eh + 1) * 512], ps[:]
                            )
                        nc.sync.dma_start(out_d[i * 128 : (i + 1) * 128, :], osb[:])
                    tasks.append(oproj_i)
                return tasks

            uos = {}
            for n in range(NNC):
                for hpp in (0, 2):
                    av_a = av_ps.tile([128, 512], F32, tag="av")
                    av_b = av_ps.tile([128, 512], F32, tag="av")
                    zz = z_ps.tile([128, 512], F32, tag="zz")
                    pend = None
                    for t in range(NT + 1):
                        cur = None
                        if t < NT:
                            ex_a = score_step(hpp, n, t)
                            ex_b = score_step(hpp + 1, n, t)
                            cur = (ex_a, ex_b)
                        if pend is not None:
                            av_pair(hpp, t - 1, av_a, pend[0])
                            av_pair(hpp + 1, t - 1, av_b, pend[1])
                            sum_pair(t - 1, zz, 0, pend[0])
                            sum_pair(t - 1, zz, 64, pend[1])
                        pend = cur
                        drip()
                    for hp, av, zrow in ((hpp, av_a, 0), (hpp + 1, av_b, 64)):
                        uo = stg.tile([128, 512], F32, tag="uo")
                        nc.vector.tensor_copy(uo[:], av[:])
                        nc.vector.tensor_copy(
                            zstage[0:1, hp * 512 : (hp + 1) * 512],
                            zz[zrow : zrow + 1, :],
                        )
                        nc.vector.tensor_copy(
                            zstage[32:33, hp * 512 : (hp + 1) * 512],
                            zz[zrow + 32 : zrow + 33, :],
                        )
                        uos[(n, hp)] = uo

                # zbuf rows 0-3 = even heads of pair hp, rows 4-7 = odd heads
                for hp in range(NM):
                    nc.sync.dma_start(
                        zbuf[hp : hp + 1, n * 512 : (n + 1) * 512],
                        zstage[0:1, hp * 512 : (hp + 1) * 512],
                    )
                    nc.sync.dma_start(
                        zbuf[4 + hp : 5 + hp, n * 512 : (n + 1) * 512],
                        zstage[32:33, hp * 512 : (hp + 1) * 512],
                    )
                nc.vector.reciprocal(
                    zrec[:, n * 512 : (n + 1) * 512], zbuf[:, n * 512 : (n + 1) * 512]
                )
                nc.vector.tensor_copy(
                    zrecc[:, n * 512 : (n + 1) * 512], zrec[:, n * 512 : (n + 1) * 512]
                )
                filler.extend(boundary(n))
            while filler:
                filler.pop(0)()

    return nc


def _make_sel():
    # zbuf row for head (2k + p//64): even heads -> row k, odd heads -> row 4+k
    sel = np.zeros((HG, 512), dtype=np.float32)
    for k in range(4):
        for p in range(128):
            r = k if p < 64 else 4 + k
            sel[r, k * 128 + p] = 1.0
    return sel


def kernel(x, Wq, Wk, Wv, Wo):
    global LAST_EXEC_TIME_NS
    _env_setup()
    from concourse.bass_utils import run_bass_kernel_spmd

    x = np.asarray(x, dtype=np.float32)
    Wq = np.asarray(Wq, dtype=np.float32)
    Wk = np.asarray(Wk, dtype=np.float32)
    Wv = np.asarray(Wv, dtype=np.float32)
    Wo = np.asarray(Wo, dtype=np.float32)

    if "nc" not in _cache:
        _cache["nc"] = _build_nc()
    nc = _cache["nc"]

    if MM_DTYPE == "bf16":
        import ml_dtypes

        cdt = ml_dtypes.bfloat16
    else:
        cdt = np.float32

    sel = _make_sel()
    in_maps = []
    for c in range(NCORES):
        b, hg = c // 2, c % 2
        r = slice(hg * DG, (hg + 1) * DG)
        in_maps.append(
            {
                "xT": np.ascontiguousarray(x[b].T).astype(cdt),
                "wqT": np.ascontiguousarray(Wq[r, :].T).astype(cdt),
                "wkT": np.ascontiguousarray(Wk[r, :].T).astype(cdt),
                "wvT": np.ascontiguousarray(Wv[r, :].T).astype(cdt),
                "woT": np.ascontiguousarray(Wo[:, r].T).astype(cdt),
                "sel": sel.astype(cdt),
            }
        )

    res = run_bass_kernel_spmd(
        nc, in_maps, core_ids=list(range(NCORES)), trace=TRACE
    )
    if TRACE:
        LAST_EXEC_TIME_NS = res.exec_time_ns

    out = np.empty((B, S, E), dtype=np.float32)
    for b in range(B):
        out[b] = res.results[2 * b]["out"] + res.results[2 * b + 1]["out"]
    return out


# revision 25
# speedup vs baseline: 1.3794x; 1.0269x over previous
"""Multi-head attention Trainium2 kernel (8 NeuronCores, Bass/Tile).

Sharding: core c -> (batch b = c//2, head-group hg = c%2). Each core computes
attention for 8 of the 16 heads of one batch element plus its partial
out-projection; the host sums the two head-group partials per batch.

Per-core layouts (host pre-transposes inputs; contraction dims on partitions):
  xT  [E=1024, S=2048]      x[b].T
  wqT/wkT/wvT [1024, 512]   W[hg_rows].T
  woT [512, 1024]           Wo[:, hg_cols].T
  sel [8, 512]              0/1 selector for softmax-denominator replication

On-chip pipeline (all fp32):
  QT = wqT.T-tiles @ xT   [512, 2048] (head-major, transposed)
  KT likewise; V natural [2048, 512] with a ones-column appended per head
  scoresT[t,s] = KT_h.T-tile @ QT_h  (K=64, two heads row-packed per PE pass)
  expT = exp(scoresT/8) on ScalarE, batched [128, 2048] over 4 psum banks
  (outT | Z) = [V_h | 1].T @ expT    (M=65 matmul: row 64 = softmax sums)
  outT_norm = outT * replicate(1/Z)  (K=8 selector matmul + DVE mult)
  out = outT_norm.T-tiles @ woT      [2048, 1024] partial
"""

import os
import sys
import types

import numpy as np

B, S, E, H = 4, 2048, 1024, 16
DK = E // H  # 64
HG = H // 2  # heads per core = 8
DG = HG * DK  # 512 projected dims per core
NCORES = 8

TRACE = bool(os.environ.get("TRN_KERNEL_TRACE"))
# matmul-operand dtype: bf16 single-pass PE (fp32 PSUM accumulation) vs
# fp32 operands (PE double-pumps each matmul -> ~2x slower)
MM_DTYPE = os.environ.get("TRN_MM_DTYPE", "bf16")
LAST_EXEC_TIME_NS = None

_cache = {}


def _env_setup():
    import antenv

    if "antenv.axon_hooks" not in sys.modules:
        mod = types.ModuleType("antenv.axon_hooks")
        mod._hook = None
        mod.set_axon_ntff_profile_hook = lambda h: setattr(mod, "_hook", h)
        mod.get_axon_ntff_profile_hook = lambda: mod._hook
        sys.modules["antenv.axon_hooks"] = mod
        antenv.axon_hooks = mod
        try:
            from trn_agent_boot.trn_boot import _ntff_profile_via_ctypes

            mod.set_axon_ntff_profile_hook(
                _ntff_profile_via_ctypes("/opt/axon/libaxon_pjrt.so")
            )
        except Exception:
            pass

    import concourse.bass_utils as bass_utils

    bass_utils.upload_artifacts = lambda tmpdir: tmpdir

    import concourse.tile as tile
    from concourse import mybir
    from concourse.vector_clock import ScopedClock

    if getattr(tile.TileContext, "_wait_split_patched", False):
        return

    MAX_WAITS = 1  # walrus on this image rejects >1 sync wait per instruction

    def _drain_and_barrier_split(self, tick_clock, wait_clock):
        probe = self.nc.sync.drain()
        wait_clock.add_sem_waits(
            probe.ins, ScopedClock({None: tick_clock.global_clock})
        )
        waits = list(probe.ins.sync_info.on_wait)
        if len(waits) > MAX_WAITS:
            num2h = {h.num: h for h in self.sems.allocated().values()}
            probe.ins.sync_info.on_wait = []
            for w in waits:
                self.nc.sync.wait_ge(num2h[w.id], w.wait_value)
            self.nc.sync.drain()
        self.nc.all_engine_barrier()
        popped = self.nc._tile_sem_poison_stack.pop()
        assert popped is self._sem_poison
        self.nc.clear_and_free_semaphores(list(self.sems.allocated().values()))
        self.nc.all_engine_barrier()

    _orig_commit = tile.TileContext._commit_instruction
    _ctr = [0]

    def _commit_split_waits(self, inst, lazy_reg_writes=True):
        si = inst.sync_info
        if (
            si is not None
            and len(si.on_wait) > MAX_WAITS
            and inst.engine != mybir.EngineType.Unassigned
        ):
            waits = list(si.on_wait)
            keep, hoist = waits[:MAX_WAITS], waits[MAX_WAITS:]
            for i in range(0, len(hoist), MAX_WAITS):
                _ctr[0] += 1
                nop = mybir.InstNoOp(name=f"waitnop-{_ctr[0]}", ins=[], outs=[])
                nop.engine = inst.engine
                nop.sync_info = mybir.SyncInfo(
                    on_wait=hoist[i : i + MAX_WAITS], on_update=[]
                )
                self.nc.register_instruction(nop, overwrite=True)
                _orig_commit(self, nop, lazy_reg_writes=False)
            inst.sync_info = mybir.SyncInfo(on_wait=keep, on_update=list(si.on_update))
        return _orig_commit(self, inst, lazy_reg_writes=lazy_reg_writes)

    tile.TileContext._drain_and_barrier = _drain_and_barrier_split
    tile.TileContext._commit_instruction = _commit_split_waits
    tile.TileContext._wait_split_patched = True

    # use the full usable SBUF on trn2 (default constant is stale)
    import concourse.tile_utils as tile_utils

    tile_utils.max_sbuf_usage = 206 * 1024

    if os.environ.get("TRN_LDW_OPT"):
        _orig_bvo = bass_utils.bir_verify_and_optimise

        def _bvo_ldwopt(*a, **kw):
            orig_run = bass_utils.run_command

            def run_patched(cmd, **rkw):
                cmd = [
                    c.replace("--enable-ldw-opt=false", "--enable-ldw-opt=true")
                    if isinstance(c, str)
                    else c
                    for c in cmd
                ]
                return orig_run(cmd, **rkw)

            bass_utils.run_command = run_patched
            try:
                return _orig_bvo(*a, **kw)
            finally:
                bass_utils.run_command = orig_run

        bass_utils.bir_verify_and_optimise = _bvo_ldwopt


def _build_nc():
    import contextlib

    import concourse.bass as bass
    import concourse.tile as tile
    from concourse import mybir

    F32 = mybir.dt.float32
    CDT = mybir.dt.bfloat16 if MM_DTYPE == "bf16" else mybir.dt.float32
    PS = bass.MemorySpace.PSUM
    AF = mybir.ActivationFunctionType

    nc = bass.Bass()
    xT_d = nc.dram_tensor("xT", [E, S], CDT, kind="ExternalInput")
    wqT_d = nc.dram_tensor("wqT", [E, DG], CDT, kind="ExternalInput")
    wkT_d = nc.dram_tensor("wkT", [E, DG], CDT, kind="ExternalInput")
    wvT_d = nc.dram_tensor("wvT", [E, DG], CDT, kind="ExternalInput")
    woT_d = nc.dram_tensor("woT", [DG, E], CDT, kind="ExternalInput")
    sel_d = nc.dram_tensor("sel", [HG, 512], CDT, kind="ExternalInput")
    out_d = nc.dram_tensor("out", [S, E], F32, kind="ExternalOutput")

    NE = E // 128  # 8 e-tiles
    NT = S // 128  # 16 t/s-tiles
    NNC = S // 512  # 4 s-chunks
    NM = DG // 128  # 4 head-pair tiles

    with tile.TileContext(nc) as tc:
        st = contextlib.ExitStack()
        with st:
            pp = st.enter_context(tc.tile_pool(name="persist", bufs=1))
            stg = st.enter_context(tc.tile_pool(name="stage", bufs=6))
            expp = st.enter_context(tc.tile_pool(name="expp", bufs=4))
            outp = st.enter_context(tc.tile_pool(name="outp", bufs=2))

            sel_sb = pp.tile([HG, 512], CDT, tag="sel")
            nc.sync.dma_start(sel_sb[:], sel_d[:])

            QT = pp.tile([128, NM * S], CDT, tag="QT")  # [128, 8192]
            KT = pp.tile([128, NM * S], CDT, tag="KT")
            Vsb = pp.tile([128, NT * DG], CDT, tag="V")  # [128, 8192]
            onorm = pp.tile([128, NM * S], CDT, tag="onorm")
            zbuf = pp.tile([HG, S], F32, tag="zbuf")
            zrec = pp.tile([HG, S], F32, tag="zrec")
            zrecc = pp.tile([HG, S], CDT, tag="zrecc")
            zstage = pp.tile([128, NM * 512], F32, tag="zstage")
            woT = pp.tile([128, NM * E], CDT, tag="woT")  # [128, 4096]
            ones = pp.tile([128, 1], CDT, tag="ones")
            nc.gpsimd.memset(ones[:], 1.0)

            xT = pp.tile([128, NE * S], CDT, tag="xT")  # [128, 16384]
            wq = pp.tile([128, NE * DG], CDT, tag="wq")
            wk = pp.tile([128, NE * DG], CDT, tag="wk")
            wv = pp.tile([128, NE * DG], CDT, tag="wv")
            for j in range(NE):
                nc.sync.dma_start(
                    xT[:, j * S : (j + 1) * S], xT_d[j * 128 : (j + 1) * 128, :]
                )
            for w_sb, w_d in ((wq, wqT_d), (wk, wkT_d), (wv, wvT_d)):
                for j in range(NE):
                    nc.sync.dma_start(
                        w_sb[:, j * DG : (j + 1) * DG],
                        w_d[j * 128 : (j + 1) * 128, :],
                    )
            for k in range(NM):
                nc.sync.dma_start(
                    woT[:, k * E : (k + 1) * E], woT_d[k * 128 : (k + 1) * 128, :]
                )

            def qk_proj(w_sb, dst, m, n):
                acc = proj_ps.tile([128, 512], F32, tag="acc")
                for j in range(NE):
                    nc.tensor.matmul(
                        acc[:],
                        w_sb[:, j * DG + m * 128 : j * DG + (m + 1) * 128],
                        xT[:, j * S + n * 512 : j * S + (n + 1) * 512],
                        start=(j == 0),
                        stop=(j == NE - 1),
                    )
                nc.vector.tensor_copy(
                    dst[:, m * S + n * 512 : m * S + (n + 1) * 512], acc[:]
                )

            def v_proj(i):
                acc = proj_ps.tile([128, 512], F32, tag="acc")
                for j in range(NE):
                    nc.tensor.matmul(
                        acc[:],
                        xT[:, j * S + i * 128 : j * S + (i + 1) * 128],
                        wv[:, j * DG : (j + 1) * DG],
                        start=(j == 0),
                        stop=(j == NE - 1),
                    )
                nc.vector.tensor_copy(Vsb[:, i * DG : (i + 1) * DG], acc[:])

            # full projection phase under a scoped, deep psum pool
            with tc.tile_pool(name="projps", bufs=3, space=PS) as proj_ps:
                for m in range(NM):
                    for n in range(NNC):
                        qk_proj(wq, QT, m, n)
                        qk_proj(wk, KT, m, n)
                for i in range(NT):
                    v_proj(i)

            sc_ps = st.enter_context(tc.tile_pool(name="scpsum", bufs=2, space=PS))
            av_ps = st.enter_context(tc.tile_pool(name="avpsum", bufs=2, space=PS))
            z_ps = st.enter_context(tc.tile_pool(name="zpsum", bufs=1, space=PS))
            ms_ps = st.enter_context(tc.tile_pool(name="miscpsum", bufs=1, space=PS))

            filler = []

            def drip():
                if filler:
                    filler.pop(0)()

            def score_step(hp, n, t):
                sc = sc_ps.tile([128, 1024], F32, tag="sc")
                nc.tensor.matmul(
                    sc[:, 0:512],
                    KT[0:64, hp * S + t * 128 : hp * S + (t + 1) * 128],
                    QT[0:64, hp * S + n * 512 : hp * S + (n + 1) * 512],
                )
                nc.tensor.matmul(
                    sc[:, 512:1024],
                    KT[64:128, hp * S + t * 128 : hp * S + (t + 1) * 128],
                    QT[64:128, hp * S + n * 512 : hp * S + (n + 1) * 512],
                )
                ex = expp.tile([128, 1024], CDT, tag="ex")
                nc.scalar.activation(ex[:], sc[:], AF.Exp, scale=0.125)
                return ex

            def av_pair(hp, t, av, ex):
                voff = t * DG
                nc.tensor.matmul(
                    av[0:64, :],
                    Vsb[:, voff + (2 * hp) * DK : voff + (2 * hp) * DK + DK],
                    ex[:, 0:512],
                    start=(t == 0),
                    stop=(t == NT - 1),
                    tile_position=(0, 0),
                    skip_group_check=True,
                )
                nc.tensor.matmul(
                    av[64:128, :],
                    Vsb[:, voff + (2 * hp + 1) * DK : voff + (2 * hp + 1) * DK + DK],
                    ex[:, 512:1024],
                    start=(t == 0),
                    stop=(t == NT - 1),
                    tile_position=(0, 64),
                    skip_group_check=True,
                )

            def sum_pair(t, zz, zrow, ex):
                nc.tensor.matmul(
                    zz[zrow : zrow + 1, :],
                    ones[:, 0:1],
                    ex[:, 0:512],
                    start=(t == 0),
                    stop=(t == NT - 1),
                    tile_position=(0, zrow),
                    skip_group_check=True,
                )
                nc.tensor.matmul(
                    zz[zrow + 32 : zrow + 33, :],
                    ones[:, 0:1],
                    ex[:, 512:1024],
                    start=(t == 0),
                    stop=(t == NT - 1),
                    tile_position=(0, zrow + 32),
                    skip_group_check=True,
                )

            def boundary(n):
                """post-chunk-n work, dripped into chunk n+1: normalize+outproj."""
                tasks = []
                for k in range(NM):
                    def norm_k(k=k, n=n):
                        rep = ms_ps.tile([128, 512], F32, tag="ms")
                        nc.tensor.matmul(
                            rep[:],
                            sel_sb[:, k * 128 : (k + 1) * 128],
                            zrecc[:, n * 512 : (n + 1) * 512],
                        )
                        nc.vector.tensor_tensor(
                            onorm[:, k * S + n * 512 : k * S + (n + 1) * 512],
                            uos[(n, k)][:],
                            rep[:],
                            mybir.AluOpType.mult,
                        )
                    tasks.append(norm_k)
                for i in range(4 * n, 4 * n + 4):
                    def oproj_i(i=i):
                        osb = outp.tile([128, E], F32, tag="osb")
                        for eh in range(2):
                            ps = ms_ps.tile([128, 512], F32, tag="ms")
                            for k in range(NM):
                                nc.tensor.matmul(
                                    ps[:],
                                    onorm[:, k * S + i * 128 : k * S + (i + 1) * 128],
                                    woT[:, k * E + eh * 512 : k * E + (eh + 1) * 512],
                                    start=(k == 0),
                                    stop=(k == NM - 1),
                                )
                            nc.vector.tensor_copy(
                                osb[:, eh * 512 : (eh + 1) * 512], ps[:]
                            )
                        nc.sync.dma_start(out_d[i * 128 : (i + 1) * 128, :], osb[:])
                    tasks.append(oproj_i)
                return tasks

            uos = {}
            for n in range(NNC):
                for hpp in (0, 2):
                    av_a = av_ps.tile([128, 512], F32, tag="av")
                    av_b = av_ps.tile([128, 512], F32, tag="av")
                    zz = z_ps.tile([128, 512], F32, tag="zz")
                    pend = None
                    for t in range(NT + 1):
                        cur = None
                        if t < NT:
                            ex_a = score_step(hpp, n, t)
                            ex_b = score_step(hpp + 1, n, t)
                            cur = (ex_a, ex_b)
                        if pend is not None:
                            av_pair(hpp, t - 1, av_a, pend[0])
                            av_pair(hpp + 1, t - 1, av_b, pend[1])
                            sum_pair(t - 1, zz, 0, pend[0])
                            sum_pair(t - 1, zz, 64, pend[1])
                        pend = cur
                        drip()
                    for hp, av, zrow in ((hpp, av_a, 0), (hpp + 1, av_b, 64)):
                        uo = stg.tile([128, 512], F32, tag="uo")
                        nc.vector.tensor_copy(uo[:], av[:])
                        nc.vector.tensor_copy(
                            zstage[0:1, hp * 512 : (hp + 1) * 512],
                            zz[zrow : zrow + 1, :],
                        )
                        nc.vector.tensor_copy(
                            zstage[32:33, hp * 512 : (hp + 1) * 512],
                            zz[zrow + 32 : zrow + 33, :],
                        )
                        uos[(n, hp)] = uo

                # zbuf rows 0-3 = even heads of pair hp, rows 4-7 = odd heads
                for hp in range(NM):
                    nc.sync.dma_start(
                        zbuf[hp : hp + 1, n * 512 : (n + 1) * 512],
                        zstage[0:1, hp * 512 : (hp + 1) * 512],
                    )
                    nc.sync.dma_start(
                        zbuf[4 + hp : 5 + hp, n * 512 : (n + 1) * 512],
                        zstage[32:33, hp * 512 : (hp + 1) * 512],
                    )
                nc.vector.reciprocal(
                    zrec[:, n * 512 : (n + 1) * 512], zbuf[:, n * 512 : (n + 1) * 512]
                )
                nc.vector.tensor_copy(
                    zrecc[:, n * 512 : (n + 1) * 512], zrec[:, n * 512 : (n + 1) * 512]
                )
                filler.extend(boundary(n))
            while filler:
                filler.pop(0)()

    return nc


def _make_sel():
    # zbuf row for head (2k + p//64): even heads -> row k, odd heads -> row 4+k
    sel = np.zeros((HG, 512), dtype=np.float32)
    for k in range(4):
        for p in range(128):
            r = k if p < 64 else 4 + k
            sel[r, k * 128 + p] = 1.0
    return sel


def kernel(x, Wq, Wk, Wv, Wo):
    global LAST_EXEC_TIME_NS
    _env_setup()
    from concourse.bass_utils import run_bass_kernel_spmd

    x = np.asarray(x, dtype=np.float32)
    Wq = np.asarray(Wq, dtype=np.float32)
    Wk = np.asarray(Wk, dtype=np.float32)
    Wv = np.asarray(Wv, dtype=np.float32)
    Wo = np.asarray(Wo, dtype=np.float32)

    if "nc" not in _cache:
        _cache["nc"] = _build_nc()
    nc = _cache["nc"]

    if MM_DTYPE == "bf16":
        import ml_dtypes

        cdt = ml_dtypes.bfloat16
    else:
        cdt = np.float32

    sel = _make_sel()
    in_maps = []
    for c in range(NCORES):
        b, hg = c // 2, c % 2
        r = slice(hg * DG, (hg + 1) * DG)
        in_maps.append(
            {
                "xT": np.ascontiguousarray(x[b].T).astype(cdt),
                "wqT": np.ascontiguousarray(Wq[r, :].T).astype(cdt),
                "wkT": np.ascontiguousarray(Wk[r, :].T).astype(cdt),
                "wvT": np.ascontiguousarray(Wv[r, :].T).astype(cdt),
                "woT": np.ascontiguousarray(Wo[:, r].T).astype(cdt),
                "sel": sel.astype(cdt),
            }
        )

    res = run_bass_kernel_spmd(
        nc, in_maps, core_ids=list(range(NCORES)), trace=TRACE
    )
    if TRACE:
        LAST_EXEC_TIME_NS = res.exec_time_ns

    out = np.empty((B, S, E), dtype=np.float32)
    for b in range(B):
        out[b] = res.results[2 * b]["out"] + res.results[2 * b + 1]["out"]
    return out


# revision 27
# speedup vs baseline: 1.4172x; 1.0274x over previous
"""Multi-head attention Trainium2 kernel (8 NeuronCores, Bass/Tile).

Sharding: core c -> (batch b = c//2, head-group hg = c%2). Each core computes
attention for 8 of the 16 heads of one batch element plus its partial
out-projection; the host sums the two head-group partials per batch.

Per-core layouts (host pre-transposes inputs; contraction dims on partitions):
  xT  [E=1024, S=2048]      x[b].T
  wqT/wkT/wvT [1024, 512]   W[hg_rows].T
  woT [512, 1024]           Wo[:, hg_cols].T
  sel [8, 512]              0/1 selector for softmax-denominator replication

On-chip pipeline (all fp32):
  QT = wqT.T-tiles @ xT   [512, 2048] (head-major, transposed)
  KT likewise; V natural [2048, 512] with a ones-column appended per head
  scoresT[t,s] = KT_h.T-tile @ QT_h  (K=64, two heads row-packed per PE pass)
  expT = exp(scoresT/8) on ScalarE, batched [128, 2048] over 4 psum banks
  (outT | Z) = [V_h | 1].T @ expT    (M=65 matmul: row 64 = softmax sums)
  outT_norm = outT * replicate(1/Z)  (K=8 selector matmul + DVE mult)
  out = outT_norm.T-tiles @ woT      [2048, 1024] partial
"""

import os
import sys
import types

import numpy as np

B, S, E, H = 4, 2048, 1024, 16
DK = E // H  # 64
HG = H // 2  # heads per core = 8
DG = HG * DK  # 512 projected dims per core
NCORES = 8

TRACE = bool(os.environ.get("TRN_KERNEL_TRACE"))
# matmul-operand dtype: bf16 single-pass PE (fp32 PSUM accumulation) vs
# fp32 operands (PE double-pumps each matmul -> ~2x slower)
MM_DTYPE = os.environ.get("TRN_MM_DTYPE", "bf16")
LAST_EXEC_TIME_NS = None

_cache = {}


def _env_setup():
    import antenv

    if "antenv.axon_hooks" not in sys.modules:
        mod = types.ModuleType("antenv.axon_hooks")
        mod._hook = None
        mod.set_axon_ntff_profile_hook = lambda h: setattr(mod, "_hook", h)
        mod.get_axon_ntff_profile_hook = lambda: mod._hook
        sys.modules["antenv.axon_hooks"] = mod
        antenv.axon_hooks = mod
        try:
            from trn_agent_boot.trn_boot import _ntff_profile_via_ctypes

            mod.set_axon_ntff_profile_hook(
                _ntff_profile_via_ctypes("/opt/axon/libaxon_pjrt.so")
            )
        except Exception:
            pass

    import concourse.bass_utils as bass_utils

    bass_utils.upload_artifacts = lambda tmpdir: tmpdir

    import concourse.tile as tile
    from concourse import mybir
    from concourse.vector_clock import ScopedClock

    if getattr(tile.TileContext, "_wait_split_patched", False):
        return

    MAX_WAITS = 1  # walrus on this image rejects >1 sync wait per instruction

    def _drain_and_barrier_split(self, tick_clock, wait_clock):
        probe = self.nc.sync.drain()
        wait_clock.add_sem_waits(
            probe.ins, ScopedClock({None: tick_clock.global_clock})
        )
        waits = list(probe.ins.sync_info.on_wait)
        if len(waits) > MAX_WAITS:
            num2h = {h.num: h for h in self.sems.allocated().values()}
            probe.ins.sync_info.on_wait = []
            for w in waits:
                self.nc.sync.wait_ge(num2h[w.id], w.wait_value)
            self.nc.sync.drain()
        self.nc.all_engine_barrier()
        popped = self.nc._tile_sem_poison_stack.pop()
        assert popped is self._sem_poison
        self.nc.clear_and_free_semaphores(list(self.sems.allocated().values()))
        self.nc.all_engine_barrier()

    _orig_commit = tile.TileContext._commit_instruction
    _ctr = [0]

    def _commit_split_waits(self, inst, lazy_reg_writes=True):
        si = inst.sync_info
        if (
            si is not None
            and len(si.on_wait) > MAX_WAITS
            and inst.engine != mybir.EngineType.Unassigned
        ):
            waits = list(si.on_wait)
            keep, hoist = waits[:MAX_WAITS], waits[MAX_WAITS:]
            for i in range(0, len(hoist), MAX_WAITS):
                _ctr[0] += 1
                nop = mybir.InstNoOp(name=f"waitnop-{_ctr[0]}", ins=[], outs=[])
                nop.engine = inst.engine
                nop.sync_info = mybir.SyncInfo(
                    on_wait=hoist[i : i + MAX_WAITS], on_update=[]
                )
                self.nc.register_instruction(nop, overwrite=True)
                _orig_commit(self, nop, lazy_reg_writes=False)
            inst.sync_info = mybir.SyncInfo(on_wait=keep, on_update=list(si.on_update))
        return _orig_commit(self, inst, lazy_reg_writes=lazy_reg_writes)

    tile.TileContext._drain_and_barrier = _drain_and_barrier_split
    tile.TileContext._commit_instruction = _commit_split_waits
    tile.TileContext._wait_split_patched = True

    # use the full usable SBUF on trn2 (default constant is stale)
    import concourse.tile_utils as tile_utils

    tile_utils.max_sbuf_usage = 206 * 1024

    if os.environ.get("TRN_LDW_OPT"):
        _orig_bvo = bass_utils.bir_verify_and_optimise

        def _bvo_ldwopt(*a, **kw):
            orig_run = bass_utils.run_command

            def run_patched(cmd, **rkw):
                cmd = [
                    c.replace("--enable-ldw-opt=false", "--enable-ldw-opt=true")
                    if isinstance(c, str)
                    else c
                    for c in cmd
                ]
                return orig_run(cmd, **rkw)

            bass_utils.run_command = run_patched
            try:
                return _orig_bvo(*a, **kw)
            finally:
                bass_utils.run_command = orig_run

        bass_utils.bir_verify_and_optimise = _bvo_ldwopt


def _build_nc():
    import contextlib

    import concourse.bass as bass
    import concourse.tile as tile
    from concourse import mybir

    F32 = mybir.dt.float32
    CDT = mybir.dt.bfloat16 if MM_DTYPE == "bf16" else mybir.dt.float32
    PS = bass.MemorySpace.PSUM
    AF = mybir.ActivationFunctionType

    nc = bass.Bass()
    xT_d = nc.dram_tensor("xT", [E, S], CDT, kind="ExternalInput")
    wqT_d = nc.dram_tensor("wqT", [E, DG], CDT, kind="ExternalInput")
    wkT_d = nc.dram_tensor("wkT", [E, DG], CDT, kind="ExternalInput")
    wvT_d = nc.dram_tensor("wvT", [E, DG], CDT, kind="ExternalInput")
    woT_d = nc.dram_tensor("woT", [DG, E], CDT, kind="ExternalInput")
    sel_d = nc.dram_tensor("sel", [HG, 512], CDT, kind="ExternalInput")
    out_d = nc.dram_tensor("out", [S, E], F32, kind="ExternalOutput")

    NE = E // 128  # 8 e-tiles
    NT = S // 128  # 16 t/s-tiles
    NNC = S // 512  # 4 s-chunks
    NM = DG // 128  # 4 head-pair tiles

    with tile.TileContext(nc) as tc:
        st = contextlib.ExitStack()
        with st:
            pp = st.enter_context(tc.tile_pool(name="persist", bufs=1))
            stg = st.enter_context(tc.tile_pool(name="stage", bufs=6))
            expp = st.enter_context(tc.tile_pool(name="expp", bufs=4))
            outp = st.enter_context(tc.tile_pool(name="outp", bufs=2))

            sel_sb = pp.tile([HG, 512], CDT, tag="sel")
            nc.sync.dma_start(sel_sb[:], sel_d[:])

            QT = pp.tile([128, NM * S], CDT, tag="QT")  # [128, 8192]
            KT = pp.tile([128, NM * S], CDT, tag="KT")
            Vsb = pp.tile([128, NT * DG], CDT, tag="V")  # [128, 8192]
            onorm = pp.tile([128, NM * S], CDT, tag="onorm")
            zbuf = pp.tile([HG, S], F32, tag="zbuf")
            zrec = pp.tile([HG, S], F32, tag="zrec")
            zrecc = pp.tile([HG, S], CDT, tag="zrecc")
            zstage = pp.tile([128, NM * 512], F32, tag="zstage")
            woT = pp.tile([128, NM * E], CDT, tag="woT")  # [128, 4096]
            ones = pp.tile([128, 1], CDT, tag="ones")
            nc.gpsimd.memset(ones[:], 1.0)

            xT = pp.tile([128, NE * S], CDT, tag="xT")  # [128, 16384]
            wq = pp.tile([128, NE * DG], CDT, tag="wq")
            wk = pp.tile([128, NE * DG], CDT, tag="wk")
            wv = pp.tile([128, NE * DG], CDT, tag="wv")
            for j in range(NE):
                nc.sync.dma_start(
                    xT[:, j * S : (j + 1) * S], xT_d[j * 128 : (j + 1) * 128, :]
                )
            for w_sb, w_d in ((wq, wqT_d), (wk, wkT_d), (wv, wvT_d)):
                for j in range(NE):
                    nc.sync.dma_start(
                        w_sb[:, j * DG : (j + 1) * DG],
                        w_d[j * 128 : (j + 1) * 128, :],
                    )
            for k in range(NM):
                nc.sync.dma_start(
                    woT[:, k * E : (k + 1) * E], woT_d[k * 128 : (k + 1) * 128, :]
                )

            def qk_proj(w_sb, dst, m, n):
                acc = proj_ps.tile([128, 512], F32, tag="acc")
                for j in range(NE):
                    nc.tensor.matmul(
                        acc[:],
                        w_sb[:, j * DG + m * 128 : j * DG + (m + 1) * 128],
                        xT[:, j * S + n * 512 : j * S + (n + 1) * 512],
                        start=(j == 0),
                        stop=(j == NE - 1),
                    )
                nc.vector.tensor_copy(
                    dst[:, m * S + n * 512 : m * S + (n + 1) * 512], acc[:]
                )

            def v_proj(i):
                acc = proj_ps.tile([128, 512], F32, tag="acc")
                for j in range(NE):
                    nc.tensor.matmul(
                        acc[:],
                        xT[:, j * S + i * 128 : j * S + (i + 1) * 128],
                        wv[:, j * DG : (j + 1) * DG],
                        start=(j == 0),
                        stop=(j == NE - 1),
                    )
                nc.vector.tensor_copy(Vsb[:, i * DG : (i + 1) * DG], acc[:])

            # full projection phase under a scoped, deep psum pool
            with tc.tile_pool(name="projps", bufs=3, space=PS) as proj_ps:
                for m in range(NM):
                    for n in range(NNC):
                        qk_proj(wq, QT, m, n)
                        qk_proj(wk, KT, m, n)
                for i in range(NT):
                    v_proj(i)

            sc_ps = st.enter_context(tc.tile_pool(name="scpsum", bufs=2, space=PS))
            av_ps = st.enter_context(tc.tile_pool(name="avpsum", bufs=2, space=PS))
            z_ps = st.enter_context(tc.tile_pool(name="zpsum", bufs=1, space=PS))
            ms_ps = st.enter_context(tc.tile_pool(name="miscpsum", bufs=1, space=PS))

            filler = []

            def drip():
                if filler:
                    filler.pop(0)()

            def score_step(hp, n, t):
                sc = sc_ps.tile([128, 1024], F32, tag="sc")
                nc.tensor.matmul(
                    sc[:, 0:512],
                    KT[0:64, hp * S + t * 128 : hp * S + (t + 1) * 128],
                    QT[0:64, hp * S + n * 512 : hp * S + (n + 1) * 512],
                )
                nc.tensor.matmul(
                    sc[:, 512:1024],
                    KT[64:128, hp * S + t * 128 : hp * S + (t + 1) * 128],
                    QT[64:128, hp * S + n * 512 : hp * S + (n + 1) * 512],
                )
                ex = expp.tile([128, 1024], CDT, tag="ex")
                nc.scalar.activation(ex[:], sc[:], AF.Exp, scale=0.125)
                return ex

            def av_pair(hp, t, av, ex):
                voff = t * DG
                nc.tensor.matmul(
                    av[0:64, :],
                    Vsb[:, voff + (2 * hp) * DK : voff + (2 * hp) * DK + DK],
                    ex[:, 0:512],
                    start=(t == 0),
                    stop=(t == NT - 1),
                    tile_position=(0, 0),
                    skip_group_check=True,
                )
                nc.tensor.matmul(
                    av[64:128, :],
                    Vsb[:, voff + (2 * hp + 1) * DK : voff + (2 * hp + 1) * DK + DK],
                    ex[:, 512:1024],
                    start=(t == 0),
                    stop=(t == NT - 1),
                    tile_position=(0, 64),
                    skip_group_check=True,
                )

            def sum_pair(t, zz, zrow, ex):
                nc.tensor.matmul(
                    zz[zrow : zrow + 1, :],
                    ones[:, 0:1],
                    ex[:, 0:512],
                    start=(t == 0),
                    stop=(t == NT - 1),
                    tile_position=(0, zrow),
                    skip_group_check=True,
                )
                nc.tensor.matmul(
                    zz[zrow + 32 : zrow + 33, :],
                    ones[:, 0:1],
                    ex[:, 512:1024],
                    start=(t == 0),
                    stop=(t == NT - 1),
                    tile_position=(0, zrow + 32),
                    skip_group_check=True,
                )

            def boundary(n):
                """post-chunk-n work, dripped into chunk n+1: normalize+outproj."""
                tasks = []
                for k in range(NM):
                    def norm_k(k=k, n=n):
                        rep = ms_ps.tile([128, 512], F32, tag="ms")
                        nc.tensor.matmul(
                            rep[:],
                            sel_sb[:, k * 128 : (k + 1) * 128],
                            zrecc[:, n * 512 : (n + 1) * 512],
                        )
                        nc.vector.tensor_tensor(
                            onorm[:, k * S + n * 512 : k * S + (n + 1) * 512],
                            uos[(n, k)][:],
                            rep[:],
                            mybir.AluOpType.mult,
                        )
                    tasks.append(norm_k)
                osbs = {}
                for i in range(4 * n, 4 * n + 4):
                    def oproj_pre(i=i):
                        osbs[i] = outp.tile([128, E], F32, tag="osb", name=f"osb{i}")
                    def oproj_eh(i=i, eh=0):
                        ps = ms_ps.tile([128, 512], F32, tag="ms")
                        for k in range(NM):
                            nc.tensor.matmul(
                                ps[:],
                                onorm[:, k * S + i * 128 : k * S + (i + 1) * 128],
                                woT[:, k * E + eh * 512 : k * E + (eh + 1) * 512],
                                start=(k == 0),
                                stop=(k == NM - 1),
                            )
                        nc.vector.tensor_copy(
                            osbs[i][:, eh * 512 : (eh + 1) * 512], ps[:]
                        )
                    def oproj_out(i=i):
                        nc.sync.dma_start(
                            out_d[i * 128 : (i + 1) * 128, :], osbs[i][:]
                        )
                    tasks.append(oproj_pre)
                    tasks.append(lambda i=i: oproj_eh(i, 0))
                    tasks.append(lambda i=i: oproj_eh(i, 1))
                    tasks.append(oproj_out)
                return tasks

            uos = {}
            for n in range(NNC):
                for hpp in (0, 2):
                    av_a = av_ps.tile([128, 512], F32, tag="av")
                    av_b = av_ps.tile([128, 512], F32, tag="av")
                    zz = z_ps.tile([128, 512], F32, tag="zz")
                    pend = None
                    for t in range(NT + 1):
                        cur = None
                        if t < NT:
                            ex_a = score_step(hpp, n, t)
                            ex_b = score_step(hpp + 1, n, t)
                            cur = (ex_a, ex_b)
                        if pend is not None:
                            av_pair(hpp, t - 1, av_a, pend[0])
                            av_pair(hpp + 1, t - 1, av_b, pend[1])
                            sum_pair(t - 1, zz, 0, pend[0])
                            sum_pair(t - 1, zz, 64, pend[1])
                        pend = cur
                        drip()
                    for hp, av, zrow in ((hpp, av_a, 0), (hpp + 1, av_b, 64)):
                        uo = stg.tile([128, 512], F32, tag="uo")
                        nc.vector.tensor_copy(uo[:], av[:])
                        nc.vector.tensor_copy(
                            zstage[0:1, hp * 512 : (hp + 1) * 512],
                            zz[zrow : zrow + 1, :],
                        )
                        nc.vector.tensor_copy(
                            zstage[32:33, hp * 512 : (hp + 1) * 512],
                            zz[zrow + 32 : zrow + 33, :],
                        )
                        uos[(n, hp)] = uo

                # zbuf rows 0-3 = even heads of pair hp, rows 4-7 = odd heads
                for hp in range(NM):
                    nc.sync.dma_start(
                        zbuf[hp : hp + 1, n * 512 : (n + 1) * 512],
                        zstage[0:1, hp * 512 : (hp + 1) * 512],
                    )
                    nc.sync.dma_start(
                        zbuf[4 + hp : 5 + hp, n * 512 : (n + 1) * 512],
                        zstage[32:33, hp * 512 : (hp + 1) * 512],
                    )
                nc.vector.reciprocal(
                    zrec[:, n * 512 : (n + 1) * 512], zbuf[:, n * 512 : (n + 1) * 512]
                )
                nc.vector.tensor_copy(
                    zrecc[:, n * 512 : (n + 1) * 512], zrec[:, n * 512 : (n + 1) * 512]
                )
                filler.extend(boundary(n))
            while filler:
                filler.pop(0)()

    return nc


def _make_sel():
    # zbuf row for head (2k + p//64): even heads -> row k, odd heads -> row 4+k
    sel = np.zeros((HG, 512), dtype=np.float32)
    for k in range(4):
        for p in range(128):
            r = k if p < 64 else 4 + k
            sel[r, k * 128 + p] = 1.0
    return sel


def kernel(x, Wq, Wk, Wv, Wo):
    global LAST_EXEC_TIME_NS
    _env_setup()
    from concourse.bass_utils import run_bass_kernel_spmd

    x = np.asarray(x, dtype=np.float32)
    Wq = np.asarray(Wq, dtype=np.float32)
    Wk = np.asarray(Wk, dtype=np.float32)
    Wv = np.asarray(Wv, dtype=np.float32)
    Wo = np.asarray(Wo, dtype=np.float32)

    if "nc" not in _cache:
        _cache["nc"] = _build_nc()
    nc = _cache["nc"]

    if MM_DTYPE == "bf16":
        import ml_dtypes

        cdt = ml_dtypes.bfloat16
    else:
        cdt = np.float32

    sel = _make_sel()
    in_maps = []
    for c in range(NCORES):
        b, hg = c // 2, c % 2
        r = slice(hg * DG, (hg + 1) * DG)
        in_maps.append(
            {
                "xT": np.ascontiguousarray(x[b].T).astype(cdt),
                "wqT": np.ascontiguousarray(Wq[r, :].T).astype(cdt),
                "wkT": np.ascontiguousarray(Wk[r, :].T).astype(cdt),
                "wvT": np.ascontiguousarray(Wv[r, :].T).astype(cdt),
                "woT": np.ascontiguousarray(Wo[:, r].T).astype(cdt),
                "sel": sel.astype(cdt),
            }
        )

    res = run_bass_kernel_spmd(
        nc, in_maps, core_ids=list(range(NCORES)), trace=TRACE
    )
    if TRACE:
        LAST_EXEC_TIME_NS = res.exec_time_ns

    out = np.empty((B, S, E), dtype=np.float32)
    for b in range(B):
        out[b] = res.results[2 * b]["out"] + res.results[2 * b + 1]["out"]
    return out


# revision 28
# speedup vs baseline: 1.4874x; 1.0495x over previous
"""Multi-head attention Trainium2 kernel (8 NeuronCores, Bass/Tile).

Sharding: core c -> (batch b = c//2, head-group hg = c%2). Each core computes
attention for 8 of the 16 heads of one batch element plus its partial
out-projection; the host sums the two head-group partials per batch.

Per-core layouts (host pre-transposes inputs; contraction dims on partitions):
  xT  [E=1024, S=2048]      x[b].T
  wqT/wkT/wvT [1024, 512]   W[hg_rows].T
  woT [512, 1024]           Wo[:, hg_cols].T
  sel [8, 512]              0/1 selector for softmax-denominator replication

On-chip pipeline (all fp32):
  QT = wqT.T-tiles @ xT   [512, 2048] (head-major, transposed)
  KT likewise; V natural [2048, 512] with a ones-column appended per head
  scoresT[t,s] = KT_h.T-tile @ QT_h  (K=64, two heads row-packed per PE pass)
  expT = exp(scoresT/8) on ScalarE, batched [128, 2048] over 4 psum banks
  (outT | Z) = [V_h | 1].T @ expT    (M=65 matmul: row 64 = softmax sums)
  outT_norm = outT * replicate(1/Z)  (K=8 selector matmul + DVE mult)
  out = outT_norm.T-tiles @ woT      [2048, 1024] partial
"""

import os
import sys
import types

import numpy as np

B, S, E, H = 4, 2048, 1024, 16
DK = E // H  # 64
HG = H // 2  # heads per core = 8
DG = HG * DK  # 512 projected dims per core
NCORES = 8

TRACE = bool(os.environ.get("TRN_KERNEL_TRACE"))
# matmul-operand dtype: bf16 single-pass PE (fp32 PSUM accumulation) vs
# fp32 operands (PE double-pumps each matmul -> ~2x slower)
MM_DTYPE = os.environ.get("TRN_MM_DTYPE", "bf16")
LAST_EXEC_TIME_NS = None

_cache = {}


def _env_setup():
    import antenv

    if "antenv.axon_hooks" not in sys.modules:
        mod = types.ModuleType("antenv.axon_hooks")
        mod._hook = None
        mod.set_axon_ntff_profile_hook = lambda h: setattr(mod, "_hook", h)
        mod.get_axon_ntff_profile_hook = lambda: mod._hook
        sys.modules["antenv.axon_hooks"] = mod
        antenv.axon_hooks = mod
        try:
            from trn_agent_boot.trn_boot import _ntff_profile_via_ctypes

            mod.set_axon_ntff_profile_hook(
                _ntff_profile_via_ctypes("/opt/axon/libaxon_pjrt.so")
            )
        except Exception:
            pass

    import concourse.bass_utils as bass_utils

    bass_utils.upload_artifacts = lambda tmpdir: tmpdir

    import concourse.tile as tile
    from concourse import mybir
    from concourse.vector_clock import ScopedClock

    if getattr(tile.TileContext, "_wait_split_patched", False):
        return

    MAX_WAITS = 1  # walrus on this image rejects >1 sync wait per instruction

    def _drain_and_barrier_split(self, tick_clock, wait_clock):
        probe = self.nc.sync.drain()
        wait_clock.add_sem_waits(
            probe.ins, ScopedClock({None: tick_clock.global_clock})
        )
        waits = list(probe.ins.sync_info.on_wait)
        if len(waits) > MAX_WAITS:
            num2h = {h.num: h for h in self.sems.allocated().values()}
            probe.ins.sync_info.on_wait = []
            for w in waits:
                self.nc.sync.wait_ge(num2h[w.id], w.wait_value)
            self.nc.sync.drain()
        self.nc.all_engine_barrier()
        popped = self.nc._tile_sem_poison_stack.pop()
        assert popped is self._sem_poison
        self.nc.clear_and_free_semaphores(list(self.sems.allocated().values()))
        self.nc.all_engine_barrier()

    _orig_commit = tile.TileContext._commit_instruction
    _ctr = [0]

    def _commit_split_waits(self, inst, lazy_reg_writes=True):
        si = inst.sync_info
        if (
            si is not None
            and len(si.on_wait) > MAX_WAITS
            and inst.engine != mybir.EngineType.Unassigned
        ):
            waits = list(si.on_wait)
            keep, hoist = waits[:MAX_WAITS], waits[MAX_WAITS:]
            for i in range(0, len(hoist), MAX_WAITS):
                _ctr[0] += 1
                nop = mybir.InstNoOp(name=f"waitnop-{_ctr[0]}", ins=[], outs=[])
                nop.engine = inst.engine
                nop.sync_info = mybir.SyncInfo(
                    on_wait=hoist[i : i + MAX_WAITS], on_update=[]
                )
                self.nc.register_instruction(nop, overwrite=True)
                _orig_commit(self, nop, lazy_reg_writes=False)
            inst.sync_info = mybir.SyncInfo(on_wait=keep, on_update=list(si.on_update))
        return _orig_commit(self, inst, lazy_reg_writes=lazy_reg_writes)

    tile.TileContext._drain_and_barrier = _drain_and_barrier_split
    tile.TileContext._commit_instruction = _commit_split_waits
    tile.TileContext._wait_split_patched = True

    # use the full usable SBUF on trn2 (default constant is stale)
    import concourse.tile_utils as tile_utils

    tile_utils.max_sbuf_usage = 206 * 1024

    if os.environ.get("TRN_LDW_OPT"):
        _orig_bvo = bass_utils.bir_verify_and_optimise

        def _bvo_ldwopt(*a, **kw):
            orig_run = bass_utils.run_command

            def run_patched(cmd, **rkw):
                cmd = [
                    c.replace("--enable-ldw-opt=false", "--enable-ldw-opt=true")
                    if isinstance(c, str)
                    else c
                    for c in cmd
                ]
                return orig_run(cmd, **rkw)

            bass_utils.run_command = run_patched
            try:
                return _orig_bvo(*a, **kw)
            finally:
                bass_utils.run_command = orig_run

        bass_utils.bir_verify_and_optimise = _bvo_ldwopt


def _build_nc():
    import contextlib

    import concourse.bass as bass
    import concourse.tile as tile
    from concourse import mybir

    F32 = mybir.dt.float32
    CDT = mybir.dt.bfloat16 if MM_DTYPE == "bf16" else mybir.dt.float32
    PS = bass.MemorySpace.PSUM
    AF = mybir.ActivationFunctionType

    nc = bass.Bass()
    xT_d = nc.dram_tensor("xT", [E, S], CDT, kind="ExternalInput")
    wqT_d = nc.dram_tensor("wqT", [E, DG], CDT, kind="ExternalInput")
    wkT_d = nc.dram_tensor("wkT", [E, DG], CDT, kind="ExternalInput")
    wvT_d = nc.dram_tensor("wvT", [E, DG], CDT, kind="ExternalInput")
    woT_d = nc.dram_tensor("woT", [DG, E], CDT, kind="ExternalInput")
    sel_d = nc.dram_tensor("sel", [HG, 512], CDT, kind="ExternalInput")
    out_d = nc.dram_tensor("out", [S, E], F32, kind="ExternalOutput")

    NE = E // 128  # 8 e-tiles
    NT = S // 128  # 16 t/s-tiles
    NNC = S // 512  # 4 s-chunks
    NM = DG // 128  # 4 head-pair tiles

    with tile.TileContext(nc) as tc:
        st = contextlib.ExitStack()
        with st:
            pp = st.enter_context(tc.tile_pool(name="persist", bufs=1))
            stg = st.enter_context(tc.tile_pool(name="stage", bufs=12))
            expp = st.enter_context(tc.tile_pool(name="expp", bufs=8))
            outp = st.enter_context(tc.tile_pool(name="outp", bufs=2))

            sel_sb = pp.tile([HG, 512], CDT, tag="sel")
            nc.sync.dma_start(sel_sb[:], sel_d[:])

            QT = pp.tile([128, NM * S], CDT, tag="QT")  # [128, 8192]
            KT = pp.tile([128, NM * S], CDT, tag="KT")
            Vsb = pp.tile([128, NT * DG], CDT, tag="V")  # [128, 8192]
            onorm = pp.tile([128, NM * S], CDT, tag="onorm")
            zbuf = pp.tile([HG, S], F32, tag="zbuf")
            zrec = pp.tile([HG, S], F32, tag="zrec")
            zrecc = pp.tile([HG, S], CDT, tag="zrecc")
            zstage = pp.tile([128, NM * 512], F32, tag="zstage")
            woT = pp.tile([128, NM * E], CDT, tag="woT")  # [128, 4096]
            ones = pp.tile([128, 1], CDT, tag="ones")
            nc.gpsimd.memset(ones[:], 1.0)

            xT = pp.tile([128, NE * S], CDT, tag="xT")  # [128, 16384]
            wq = pp.tile([128, NE * DG], CDT, tag="wq")
            wk = pp.tile([128, NE * DG], CDT, tag="wk")
            wv = pp.tile([128, NE * DG], CDT, tag="wv")
            for j in range(NE):
                nc.sync.dma_start(
                    xT[:, j * S : (j + 1) * S], xT_d[j * 128 : (j + 1) * 128, :]
                )
            for w_sb, w_d in ((wq, wqT_d), (wk, wkT_d), (wv, wvT_d)):
                for j in range(NE):
                    nc.sync.dma_start(
                        w_sb[:, j * DG : (j + 1) * DG],
                        w_d[j * 128 : (j + 1) * 128, :],
                    )
            for k in range(NM):
                nc.sync.dma_start(
                    woT[:, k * E : (k + 1) * E], woT_d[k * 128 : (k + 1) * 128, :]
                )

            def qk_proj(w_sb, dst, m, n):
                acc = proj_ps.tile([128, 512], F32, tag="acc")
                for j in range(NE):
                    nc.tensor.matmul(
                        acc[:],
                        w_sb[:, j * DG + m * 128 : j * DG + (m + 1) * 128],
                        xT[:, j * S + n * 512 : j * S + (n + 1) * 512],
                        start=(j == 0),
                        stop=(j == NE - 1),
                    )
                nc.vector.tensor_copy(
                    dst[:, m * S + n * 512 : m * S + (n + 1) * 512], acc[:]
                )

            def v_proj(i):
                acc = proj_ps.tile([128, 512], F32, tag="acc")
                for j in range(NE):
                    nc.tensor.matmul(
                        acc[:],
                        xT[:, j * S + i * 128 : j * S + (i + 1) * 128],
                        wv[:, j * DG : (j + 1) * DG],
                        start=(j == 0),
                        stop=(j == NE - 1),
                    )
                nc.vector.tensor_copy(Vsb[:, i * DG : (i + 1) * DG], acc[:])

            # full projection phase under a scoped, deep psum pool
            with tc.tile_pool(name="projps", bufs=3, space=PS) as proj_ps:
                for m in range(NM):
                    for n in range(NNC):
                        qk_proj(wq, QT, m, n)
                        qk_proj(wk, KT, m, n)
                for i in range(NT):
                    v_proj(i)

            sc_ps = st.enter_context(tc.tile_pool(name="scpsum", bufs=2, space=PS))
            av_ps = st.enter_context(tc.tile_pool(name="avpsum", bufs=2, space=PS))
            z_ps = st.enter_context(tc.tile_pool(name="zpsum", bufs=1, space=PS))
            ms_ps = st.enter_context(tc.tile_pool(name="miscpsum", bufs=1, space=PS))

            filler = []

            def drip():
                if filler:
                    filler.pop(0)()

            def score_step(hp, n, t):
                sc = sc_ps.tile([128, 1024], F32, tag="sc")
                nc.tensor.matmul(
                    sc[:, 0:512],
                    KT[0:64, hp * S + t * 128 : hp * S + (t + 1) * 128],
                    QT[0:64, hp * S + n * 512 : hp * S + (n + 1) * 512],
                )
                nc.tensor.matmul(
                    sc[:, 512:1024],
                    KT[64:128, hp * S + t * 128 : hp * S + (t + 1) * 128],
                    QT[64:128, hp * S + n * 512 : hp * S + (n + 1) * 512],
                )
                ex = expp.tile([128, 1024], CDT, tag="ex")
                nc.scalar.activation(ex[:], sc[:], AF.Exp, scale=0.125)
                return ex

            def av_pair(hp, t, av, ex):
                voff = t * DG
                nc.tensor.matmul(
                    av[0:64, :],
                    Vsb[:, voff + (2 * hp) * DK : voff + (2 * hp) * DK + DK],
                    ex[:, 0:512],
                    start=(t == 0),
                    stop=(t == NT - 1),
                    tile_position=(0, 0),
                    skip_group_check=True,
                )
                nc.tensor.matmul(
                    av[64:128, :],
                    Vsb[:, voff + (2 * hp + 1) * DK : voff + (2 * hp + 1) * DK + DK],
                    ex[:, 512:1024],
                    start=(t == 0),
                    stop=(t == NT - 1),
                    tile_position=(0, 64),
                    skip_group_check=True,
                )

            def sum_pair(t, zz, zrow, ex):
                nc.tensor.matmul(
                    zz[zrow : zrow + 1, :],
                    ones[:, 0:1],
                    ex[:, 0:512],
                    start=(t == 0),
                    stop=(t == NT - 1),
                    tile_position=(0, zrow),
                    skip_group_check=True,
                )
                nc.tensor.matmul(
                    zz[zrow + 32 : zrow + 33, :],
                    ones[:, 0:1],
                    ex[:, 512:1024],
                    start=(t == 0),
                    stop=(t == NT - 1),
                    tile_position=(0, zrow + 32),
                    skip_group_check=True,
                )

            def boundary(n):
                """post-chunk-n work, dripped into chunk n+1: normalize+outproj."""
                tasks = []
                for k in range(NM):
                    def norm_k(k=k, n=n):
                        rep = ms_ps.tile([128, 512], F32, tag="ms")
                        nc.tensor.matmul(
                            rep[:],
                            sel_sb[:, k * 128 : (k + 1) * 128],
                            zrecc[:, n * 512 : (n + 1) * 512],
                        )
                        nc.vector.tensor_tensor(
                            onorm[:, k * S + n * 512 : k * S + (n + 1) * 512],
                            uos[(n, k)][:],
                            rep[:],
                            mybir.AluOpType.mult,
                        )
                    tasks.append(norm_k)
                osbs = {}
                for i in range(4 * n, 4 * n + 4):
                    def oproj_pre(i=i):
                        osbs[i] = outp.tile([128, E], F32, tag="osb", name=f"osb{i}")
                    def oproj_eh(i=i, eh=0):
                        ps = ms_ps.tile([128, 512], F32, tag="ms")
                        for k in range(NM):
                            nc.tensor.matmul(
                                ps[:],
                                onorm[:, k * S + i * 128 : k * S + (i + 1) * 128],
                                woT[:, k * E + eh * 512 : k * E + (eh + 1) * 512],
                                start=(k == 0),
                                stop=(k == NM - 1),
                            )
                        nc.vector.tensor_copy(
                            osbs[i][:, eh * 512 : (eh + 1) * 512], ps[:]
                        )
                    def oproj_out(i=i):
                        nc.sync.dma_start(
                            out_d[i * 128 : (i + 1) * 128, :], osbs[i][:]
                        )
                    tasks.append(oproj_pre)
                    tasks.append(lambda i=i: oproj_eh(i, 0))
                    tasks.append(lambda i=i: oproj_eh(i, 1))
                    tasks.append(oproj_out)
                return tasks

            uos = {}
            for n in range(NNC):
                for hpp in (0, 2):
                    av_a = av_ps.tile([128, 512], F32, tag="av")
                    av_b = av_ps.tile([128, 512], F32, tag="av")
                    zz = z_ps.tile([128, 512], F32, tag="zz")
                    DLY = 2  # av/sums trail scores/exp by 2 steps
                    pend = []
                    for t in range(NT + DLY):
                        if t < NT:
                            ex_a = score_step(hpp, n, t)
                            ex_b = score_step(hpp + 1, n, t)
                            pend.append((t, ex_a, ex_b))
                        if len(pend) > DLY or t >= NT:
                            pt, pa, pb = pend.pop(0)
                            av_pair(hpp, pt, av_a, pa)
                            av_pair(hpp + 1, pt, av_b, pb)
                            sum_pair(pt, zz, 0, pa)
                            sum_pair(pt, zz, 64, pb)
                        if 4 <= t < NT:
                            drip()
                    for hp, av, zrow in ((hpp, av_a, 0), (hpp + 1, av_b, 64)):
                        uo = stg.tile([128, 512], F32, tag="uo")
                        nc.vector.tensor_copy(uo[:], av[:])
                        nc.vector.tensor_copy(
                            zstage[0:1, hp * 512 : (hp + 1) * 512],
                            zz[zrow : zrow + 1, :],
                        )
                        nc.vector.tensor_copy(
                            zstage[32:33, hp * 512 : (hp + 1) * 512],
                            zz[zrow + 32 : zrow + 33, :],
                        )
                        uos[(n, hp)] = uo

                # zbuf rows 0-3 = even heads of pair hp, rows 4-7 = odd heads
                for hp in range(NM):
                    nc.sync.dma_start(
                        zbuf[hp : hp + 1, n * 512 : (n + 1) * 512],
                        zstage[0:1, hp * 512 : (hp + 1) * 512],
                    )
                    nc.sync.dma_start(
                        zbuf[4 + hp : 5 + hp, n * 512 : (n + 1) * 512],
                        zstage[32:33, hp * 512 : (hp + 1) * 512],
                    )
                nc.vector.reciprocal(
                    zrec[:, n * 512 : (n + 1) * 512], zbuf[:, n * 512 : (n + 1) * 512]
                )
                nc.vector.tensor_copy(
                    zrecc[:, n * 512 : (n + 1) * 512], zrec[:, n * 512 : (n + 1) * 512]
                )
                filler.extend(boundary(n))
            while filler:
                filler.pop(0)()

    return nc


def _make_sel():
    # zbuf row for head (2k + p//64): even heads -> row k, odd heads -> row 4+k
    sel = np.zeros((HG, 512), dtype=np.float32)
    for k in range(4):
        for p in range(128):
            r = k if p < 64 else 4 + k
            sel[r, k * 128 + p] = 1.0
    return sel


def kernel(x, Wq, Wk, Wv, Wo):
    global LAST_EXEC_TIME_NS
    _env_setup()
    from concourse.bass_utils import run_bass_kernel_spmd

    x = np.asarray(x, dtype=np.float32)
    Wq = np.asarray(Wq, dtype=np.float32)
    Wk = np.asarray(Wk, dtype=np.float32)
    Wv = np.asarray(Wv, dtype=np.float32)
    Wo = np.asarray(Wo, dtype=np.float32)

    if "nc" not in _cache:
        _cache["nc"] = _build_nc()
    nc = _cache["nc"]

    if MM_DTYPE == "bf16":
        import ml_dtypes

        cdt = ml_dtypes.bfloat16
    else:
        cdt = np.float32

    sel = _make_sel()
    in_maps = []
    for c in range(NCORES):
        b, hg = c // 2, c % 2
        r = slice(hg * DG, (hg + 1) * DG)
        in_maps.append(
            {
                "xT": np.ascontiguousarray(x[b].T).astype(cdt),
                "wqT": np.ascontiguousarray(Wq[r, :].T).astype(cdt),
                "wkT": np.ascontiguousarray(Wk[r, :].T).astype(cdt),
                "wvT": np.ascontiguousarray(Wv[r, :].T).astype(cdt),
                "woT": np.ascontiguousarray(Wo[:, r].T).astype(cdt),
                "sel": sel.astype(cdt),
            }
        )

    res = run_bass_kernel_spmd(
        nc, in_maps, core_ids=list(range(NCORES)), trace=TRACE
    )
    if TRACE:
        LAST_EXEC_TIME_NS = res.exec_time_ns

    out = np.empty((B, S, E), dtype=np.float32)
    for b in range(B):
        out[b] = res.results[2 * b]["out"] + res.results[2 * b + 1]["out"]
    return out


# revision 29
# speedup vs baseline: 1.5094x; 1.0148x over previous
"""Multi-head attention Trainium2 kernel (8 NeuronCores, Bass/Tile).

Sharding: core c -> (batch b = c//2, head-group hg = c%2). Each core computes
attention for 8 of the 16 heads of one batch element plus its partial
out-projection; the host sums the two head-group partials per batch.

Per-core layouts (host pre-transposes inputs; contraction dims on partitions):
  xT  [E=1024, S=2048]      x[b].T
  wqT/wkT/wvT [1024, 512]   W[hg_rows].T
  woT [512, 1024]           Wo[:, hg_cols].T
  sel [8, 512]              0/1 selector for softmax-denominator replication

On-chip pipeline (all fp32):
  QT = wqT.T-tiles @ xT   [512, 2048] (head-major, transposed)
  KT likewise; V natural [2048, 512] with a ones-column appended per head
  scoresT[t,s] = KT_h.T-tile @ QT_h  (K=64, two heads row-packed per PE pass)
  expT = exp(scoresT/8) on ScalarE, batched [128, 2048] over 4 psum banks
  (outT | Z) = [V_h | 1].T @ expT    (M=65 matmul: row 64 = softmax sums)
  outT_norm = outT * replicate(1/Z)  (K=8 selector matmul + DVE mult)
  out = outT_norm.T-tiles @ woT      [2048, 1024] partial
"""

import os
import sys
import types

import numpy as np

B, S, E, H = 4, 2048, 1024, 16
DK = E // H  # 64
HG = H // 2  # heads per core = 8
DG = HG * DK  # 512 projected dims per core
NCORES = 8

TRACE = bool(os.environ.get("TRN_KERNEL_TRACE"))
# matmul-operand dtype: bf16 single-pass PE (fp32 PSUM accumulation) vs
# fp32 operands (PE double-pumps each matmul -> ~2x slower)
MM_DTYPE = os.environ.get("TRN_MM_DTYPE", "bf16")
LAST_EXEC_TIME_NS = None

_cache = {}


def _env_setup():
    import antenv

    if "antenv.axon_hooks" not in sys.modules:
        mod = types.ModuleType("antenv.axon_hooks")
        mod._hook = None
        mod.set_axon_ntff_profile_hook = lambda h: setattr(mod, "_hook", h)
        mod.get_axon_ntff_profile_hook = lambda: mod._hook
        sys.modules["antenv.axon_hooks"] = mod
        antenv.axon_hooks = mod
        try:
            from trn_agent_boot.trn_boot import _ntff_profile_via_ctypes

            mod.set_axon_ntff_profile_hook(
                _ntff_profile_via_ctypes("/opt/axon/libaxon_pjrt.so")
            )
        except Exception:
            pass

    import concourse.bass_utils as bass_utils

    bass_utils.upload_artifacts = lambda tmpdir: tmpdir

    import concourse.tile as tile
    from concourse import mybir
    from concourse.vector_clock import ScopedClock

    if getattr(tile.TileContext, "_wait_split_patched", False):
        return

    MAX_WAITS = 1  # walrus on this image rejects >1 sync wait per instruction

    def _drain_and_barrier_split(self, tick_clock, wait_clock):
        probe = self.nc.sync.drain()
        wait_clock.add_sem_waits(
            probe.ins, ScopedClock({None: tick_clock.global_clock})
        )
        waits = list(probe.ins.sync_info.on_wait)
        if len(waits) > MAX_WAITS:
            num2h = {h.num: h for h in self.sems.allocated().values()}
            probe.ins.sync_info.on_wait = []
            for w in waits:
                self.nc.sync.wait_ge(num2h[w.id], w.wait_value)
            self.nc.sync.drain()
        self.nc.all_engine_barrier()
        popped = self.nc._tile_sem_poison_stack.pop()
        assert popped is self._sem_poison
        self.nc.clear_and_free_semaphores(list(self.sems.allocated().values()))
        self.nc.all_engine_barrier()

    _orig_commit = tile.TileContext._commit_instruction
    _ctr = [0]

    def _commit_split_waits(self, inst, lazy_reg_writes=True):
        si = inst.sync_info
        if (
            si is not None
            and len(si.on_wait) > MAX_WAITS
            and inst.engine != mybir.EngineType.Unassigned
        ):
            waits = list(si.on_wait)
            keep, hoist = waits[:MAX_WAITS], waits[MAX_WAITS:]
            for i in range(0, len(hoist), MAX_WAITS):
                _ctr[0] += 1
                nop = mybir.InstNoOp(name=f"waitnop-{_ctr[0]}", ins=[], outs=[])
                nop.engine = inst.engine
                nop.sync_info = mybir.SyncInfo(
                    on_wait=hoist[i : i + MAX_WAITS], on_update=[]
                )
                self.nc.register_instruction(nop, overwrite=True)
                _orig_commit(self, nop, lazy_reg_writes=False)
            inst.sync_info = mybir.SyncInfo(on_wait=keep, on_update=list(si.on_update))
        return _orig_commit(self, inst, lazy_reg_writes=lazy_reg_writes)

    tile.TileContext._drain_and_barrier = _drain_and_barrier_split
    tile.TileContext._commit_instruction = _commit_split_waits
    tile.TileContext._wait_split_patched = True

    # use the full usable SBUF on trn2 (default constant is stale)
    import concourse.tile_utils as tile_utils

    tile_utils.max_sbuf_usage = 206 * 1024

    if os.environ.get("TRN_LDW_OPT"):
        _orig_bvo = bass_utils.bir_verify_and_optimise

        def _bvo_ldwopt(*a, **kw):
            orig_run = bass_utils.run_command

            def run_patched(cmd, **rkw):
                cmd = [
                    c.replace("--enable-ldw-opt=false", "--enable-ldw-opt=true")
                    if isinstance(c, str)
                    else c
                    for c in cmd
                ]
                return orig_run(cmd, **rkw)

            bass_utils.run_command = run_patched
            try:
                return _orig_bvo(*a, **kw)
            finally:
                bass_utils.run_command = orig_run

        bass_utils.bir_verify_and_optimise = _bvo_ldwopt


def _build_nc():
    import contextlib

    import concourse.bass as bass
    import concourse.tile as tile
    from concourse import mybir

    F32 = mybir.dt.float32
    CDT = mybir.dt.bfloat16 if MM_DTYPE == "bf16" else mybir.dt.float32
    PS = bass.MemorySpace.PSUM
    AF = mybir.ActivationFunctionType

    nc = bass.Bass()
    xT_d = nc.dram_tensor("xT", [E, S], CDT, kind="ExternalInput")
    wqT_d = nc.dram_tensor("wqT", [E, DG], CDT, kind="ExternalInput")
    wkT_d = nc.dram_tensor("wkT", [E, DG], CDT, kind="ExternalInput")
    wvT_d = nc.dram_tensor("wvT", [E, DG], CDT, kind="ExternalInput")
    woT_d = nc.dram_tensor("woT", [DG, E], CDT, kind="ExternalInput")
    sel_d = nc.dram_tensor("sel", [HG, 512], CDT, kind="ExternalInput")
    out_d = nc.dram_tensor("out", [S, E], F32, kind="ExternalOutput")

    NE = E // 128  # 8 e-tiles
    NT = S // 128  # 16 t/s-tiles
    NNC = S // 512  # 4 s-chunks
    NM = DG // 128  # 4 head-pair tiles

    with tile.TileContext(nc) as tc:
        st = contextlib.ExitStack()
        with st:
            pp = st.enter_context(tc.tile_pool(name="persist", bufs=1))
            stg = st.enter_context(tc.tile_pool(name="stage", bufs=12))
            expp = st.enter_context(tc.tile_pool(name="expp", bufs=8))
            outp = st.enter_context(tc.tile_pool(name="outp", bufs=2))

            sel_sb = pp.tile([HG, 512], CDT, tag="sel")
            nc.sync.dma_start(sel_sb[:], sel_d[:])

            QT = pp.tile([128, NM * S], CDT, tag="QT")  # [128, 8192]
            KT = pp.tile([128, NM * S], CDT, tag="KT")
            Vsb = pp.tile([128, NT * DG], CDT, tag="V")  # [128, 8192]
            onorm = pp.tile([128, NM * S], CDT, tag="onorm")
            zbuf = pp.tile([HG, S], F32, tag="zbuf")
            zrec = pp.tile([HG, S], F32, tag="zrec")
            zrecc = pp.tile([HG, S], CDT, tag="zrecc")
            zstage = pp.tile([128, NM * 512], F32, tag="zstage")
            woT = pp.tile([128, NM * E], CDT, tag="woT")  # [128, 4096]
            ones = pp.tile([128, 1], CDT, tag="ones")
            nc.gpsimd.memset(ones[:], 1.0)

            xT = pp.tile([128, NE * S], CDT, tag="xT")  # [128, 16384]
            wq = pp.tile([128, NE * DG], CDT, tag="wq")
            wk = pp.tile([128, NE * DG], CDT, tag="wk")
            wv = pp.tile([128, NE * DG], CDT, tag="wv")
            for j in range(NE):
                nc.sync.dma_start(
                    xT[:, j * S : (j + 1) * S], xT_d[j * 128 : (j + 1) * 128, :]
                )
            for w_sb, w_d in ((wq, wqT_d), (wk, wkT_d), (wv, wvT_d)):
                for j in range(NE):
                    nc.sync.dma_start(
                        w_sb[:, j * DG : (j + 1) * DG],
                        w_d[j * 128 : (j + 1) * 128, :],
                    )
            for k in range(NM):
                nc.sync.dma_start(
                    woT[:, k * E : (k + 1) * E], woT_d[k * 128 : (k + 1) * 128, :]
                )

            def qk_proj(w_sb, dst, m, n):
                acc = proj_ps.tile([128, 512], F32, tag="acc")
                for j in range(NE):
                    nc.tensor.matmul(
                        acc[:],
                        w_sb[:, j * DG + m * 128 : j * DG + (m + 1) * 128],
                        xT[:, j * S + n * 512 : j * S + (n + 1) * 512],
                        start=(j == 0),
                        stop=(j == NE - 1),
                    )
                nc.vector.tensor_copy(
                    dst[:, m * S + n * 512 : m * S + (n + 1) * 512], acc[:]
                )

            def v_proj(i):
                acc = proj_ps.tile([128, 512], F32, tag="acc")
                for j in range(NE):
                    nc.tensor.matmul(
                        acc[:],
                        xT[:, j * S + i * 128 : j * S + (i + 1) * 128],
                        wv[:, j * DG : (j + 1) * DG],
                        start=(j == 0),
                        stop=(j == NE - 1),
                    )
                nc.vector.tensor_copy(Vsb[:, i * DG : (i + 1) * DG], acc[:])

            # projection phase under a scoped, deep psum pool; the V-tail
            # drips into the first attention group (consumed two steps later)
            with tc.tile_pool(name="projps", bufs=3, space=PS) as proj_ps:
                for m in range(NM):
                    for n in range(NNC):
                        qk_proj(wq, QT, m, n)
                        qk_proj(wk, KT, m, n)
                for i in range(4):
                    v_proj(i)

            sc_ps = st.enter_context(tc.tile_pool(name="scpsum", bufs=2, space=PS))
            av_ps = st.enter_context(tc.tile_pool(name="avpsum", bufs=2, space=PS))
            z_ps = st.enter_context(tc.tile_pool(name="zpsum", bufs=1, space=PS))
            ms_ps = st.enter_context(tc.tile_pool(name="miscpsum", bufs=1, space=PS))

            filler = []

            def v_proj_ms(i):
                acc = ms_ps.tile([128, 512], F32, tag="ms")
                for j in range(NE):
                    nc.tensor.matmul(
                        acc[:],
                        xT[:, j * S + i * 128 : j * S + (i + 1) * 128],
                        wv[:, j * DG : (j + 1) * DG],
                        start=(j == 0),
                        stop=(j == NE - 1),
                    )
                nc.vector.tensor_copy(Vsb[:, i * DG : (i + 1) * DG], acc[:])

            for i in range(4, NT):
                filler.append(lambda i=i: v_proj_ms(i))

            def drip():
                if filler:
                    filler.pop(0)()

            def score_step(hp, n, t):
                sc = sc_ps.tile([128, 1024], F32, tag="sc")
                nc.tensor.matmul(
                    sc[:, 0:512],
                    KT[0:64, hp * S + t * 128 : hp * S + (t + 1) * 128],
                    QT[0:64, hp * S + n * 512 : hp * S + (n + 1) * 512],
                )
                nc.tensor.matmul(
                    sc[:, 512:1024],
                    KT[64:128, hp * S + t * 128 : hp * S + (t + 1) * 128],
                    QT[64:128, hp * S + n * 512 : hp * S + (n + 1) * 512],
                )
                ex = expp.tile([128, 1024], CDT, tag="ex")
                nc.scalar.activation(ex[:], sc[:], AF.Exp, scale=0.125)
                return ex

            def av_pair(hp, t, av, ex):
                voff = t * DG
                nc.tensor.matmul(
                    av[0:64, :],
                    Vsb[:, voff + (2 * hp) * DK : voff + (2 * hp) * DK + DK],
                    ex[:, 0:512],
                    start=(t == 0),
                    stop=(t == NT - 1),
                    tile_position=(0, 0),
                    skip_group_check=True,
                )
                nc.tensor.matmul(
                    av[64:128, :],
                    Vsb[:, voff + (2 * hp + 1) * DK : voff + (2 * hp + 1) * DK + DK],
                    ex[:, 512:1024],
                    start=(t == 0),
                    stop=(t == NT - 1),
                    tile_position=(0, 64),
                    skip_group_check=True,
                )

            def sum_pair(t, zz, zrow, ex):
                nc.tensor.matmul(
                    zz[zrow : zrow + 1, :],
                    ones[:, 0:1],
                    ex[:, 0:512],
                    start=(t == 0),
                    stop=(t == NT - 1),
                    tile_position=(0, zrow),
                    skip_group_check=True,
                )
                nc.tensor.matmul(
                    zz[zrow + 32 : zrow + 33, :],
                    ones[:, 0:1],
                    ex[:, 512:1024],
                    start=(t == 0),
                    stop=(t == NT - 1),
                    tile_position=(0, zrow + 32),
                    skip_group_check=True,
                )

            def boundary(n):
                """post-chunk-n work, dripped into chunk n+1: normalize+outproj."""
                tasks = []
                for k in range(NM):
                    def norm_k(k=k, n=n):
                        rep = ms_ps.tile([128, 512], F32, tag="ms")
                        nc.tensor.matmul(
                            rep[:],
                            sel_sb[:, k * 128 : (k + 1) * 128],
                            zrecc[:, n * 512 : (n + 1) * 512],
                        )
                        nc.vector.tensor_tensor(
                            onorm[:, k * S + n * 512 : k * S + (n + 1) * 512],
                            uos[(n, k)][:],
                            rep[:],
                            mybir.AluOpType.mult,
                        )
                    tasks.append(norm_k)
                osbs = {}
                for i in range(4 * n, 4 * n + 4):
                    def oproj_pre(i=i):
                        osbs[i] = outp.tile([128, E], F32, tag="osb", name=f"osb{i}")
                    def oproj_eh(i=i, eh=0):
                        ps = ms_ps.tile([128, 512], F32, tag="ms")
                        for k in range(NM):
                            nc.tensor.matmul(
                                ps[:],
                                onorm[:, k * S + i * 128 : k * S + (i + 1) * 128],
                                woT[:, k * E + eh * 512 : k * E + (eh + 1) * 512],
                                start=(k == 0),
                                stop=(k == NM - 1),
                            )
                        nc.vector.tensor_copy(
                            osbs[i][:, eh * 512 : (eh + 1) * 512], ps[:]
                        )
                    def oproj_out(i=i):
                        nc.sync.dma_start(
                            out_d[i * 128 : (i + 1) * 128, :], osbs[i][:]
                        )
                    tasks.append(oproj_pre)
                    tasks.append(lambda i=i: oproj_eh(i, 0))
                    tasks.append(lambda i=i: oproj_eh(i, 1))
                    tasks.append(oproj_out)
                return tasks

            uos = {}
            for n in range(NNC):
                for hpp in (0, 2):
                    av_a = av_ps.tile([128, 512], F32, tag="av")
                    av_b = av_ps.tile([128, 512], F32, tag="av")
                    zz = z_ps.tile([128, 512], F32, tag="zz")
                    DLY = 2  # av/sums trail scores/exp by 2 steps
                    pend = []
                    for t in range(NT + DLY):
                        if t < NT:
                            ex_a = score_step(hpp, n, t)
                            ex_b = score_step(hpp + 1, n, t)
                            pend.append((t, ex_a, ex_b))
                        if len(pend) > DLY or t >= NT:
                            pt, pa, pb = pend.pop(0)
                            av_pair(hpp, pt, av_a, pa)
                            av_pair(hpp + 1, pt, av_b, pb)
                            sum_pair(pt, zz, 0, pa)
                            sum_pair(pt, zz, 64, pb)
                        if 4 <= t < NT:
                            drip()
                    for hp, av, zrow in ((hpp, av_a, 0), (hpp + 1, av_b, 64)):
                        uo = stg.tile([128, 512], F32, tag="uo")
                        nc.vector.tensor_copy(uo[:], av[:])
                        nc.vector.tensor_copy(
                            zstage[0:1, hp * 512 : (hp + 1) * 512],
                            zz[zrow : zrow + 1, :],
                        )
                        nc.vector.tensor_copy(
                            zstage[32:33, hp * 512 : (hp + 1) * 512],
                            zz[zrow + 32 : zrow + 33, :],
                        )
                        uos[(n, hp)] = uo

                # zbuf rows 0-3 = even heads of pair hp, rows 4-7 = odd heads
                for hp in range(NM):
                    nc.sync.dma_start(
                        zbuf[hp : hp + 1, n * 512 : (n + 1) * 512],
                        zstage[0:1, hp * 512 : (hp + 1) * 512],
                    )
                    nc.sync.dma_start(
                        zbuf[4 + hp : 5 + hp, n * 512 : (n + 1) * 512],
                        zstage[32:33, hp * 512 : (hp + 1) * 512],
                    )
                nc.vector.reciprocal(
                    zrec[:, n * 512 : (n + 1) * 512], zbuf[:, n * 512 : (n + 1) * 512]
                )
                nc.vector.tensor_copy(
                    zrecc[:, n * 512 : (n + 1) * 512], zrec[:, n * 512 : (n + 1) * 512]
                )
                filler.extend(boundary(n))
            while filler:
                filler.pop(0)()

    return nc


def _make_sel():
    # zbuf row for head (2k + p//64): even heads -> row k, odd heads -> row 4+k
    sel = np.zeros((HG, 512), dtype=np.float32)
    for k in range(4):
        for p in range(128):
            r = k if p < 64 else 4 + k
            sel[r, k * 128 + p] = 1.0
    return sel


def kernel(x, Wq, Wk, Wv, Wo):
    global LAST_EXEC_TIME_NS
    _env_setup()
    from concourse.bass_utils import run_bass_kernel_spmd

    x = np.asarray(x, dtype=np.float32)
    Wq = np.asarray(Wq, dtype=np.float32)
    Wk = np.asarray(Wk, dtype=np.float32)
    Wv = np.asarray(Wv, dtype=np.float32)
    Wo = np.asarray(Wo, dtype=np.float32)

    if "nc" not in _cache:
        _cache["nc"] = _build_nc()
    nc = _cache["nc"]

    if MM_DTYPE == "bf16":
        import ml_dtypes

        cdt = ml_dtypes.bfloat16
    else:
        cdt = np.float32

    sel = _make_sel()
    in_maps = []
    for c in range(NCORES):
        b, hg = c // 2, c % 2
        r = slice(hg * DG, (hg + 1) * DG)
        in_maps.append(
            {
                "xT": np.ascontiguousarray(x[b].T).astype(cdt),
                "wqT": np.ascontiguousarray(Wq[r, :].T).astype(cdt),
                "wkT": np.ascontiguousarray(Wk[r, :].T).astype(cdt),
                "wvT": np.ascontiguousarray(Wv[r, :].T).astype(cdt),
                "woT": np.ascontiguousarray(Wo[:, r].T).astype(cdt),
                "sel": sel.astype(cdt),
            }
        )

    res = run_bass_kernel_spmd(
        nc, in_maps, core_ids=list(range(NCORES)), trace=TRACE
    )
    if TRACE:
        LAST_EXEC_TIME_NS = res.exec_time_ns

    out = np.empty((B, S, E), dtype=np.float32)
    for b in range(B):
        out[b] = res.results[2 * b]["out"] + res.results[2 * b + 1]["out"]
    return out


# revision 30
# speedup vs baseline: 1.5108x; 1.0010x over previous
"""Multi-head attention Trainium2 kernel (8 NeuronCores, Bass/Tile).

Sharding: core c -> (batch b = c//2, head-group hg = c%2). Each core computes
attention for 8 of the 16 heads of one batch element plus its partial
out-projection; the host sums the two head-group partials per batch.

Per-core layouts (host pre-transposes inputs; contraction dims on partitions):
  xT  [E=1024, S=2048]      x[b].T
  wqT/wkT/wvT [1024, 512]   W[hg_rows].T
  woT [512, 1024]           Wo[:, hg_cols].T
  sel [8, 512]              0/1 selector for softmax-denominator replication

On-chip pipeline (bf16 matmul operands, fp32 PSUM accumulation/softmax):
  QT = wqT.T-tiles @ xT   [512, 2048] (head-major, transposed)
  KT likewise; V natural [2048, 512]
  scoresT[t,s] = KT_h.T-tile @ QT_h   (K=64; two heads row-packed -> concurrent)
  expT = exp(scoresT/8) on ScalarE    ([128,1024] psum->sbuf per t-tile)
  outT = V_h.T @ expT                 (col-packed pair -> concurrent)
  Z    = ones.T @ expT                (col-tiled M=1 pair -> concurrent)
  outT_norm = outT * replicate(1/Z)   (K=8 selector matmul + DVE mult)
  out = outT_norm.T-tiles @ woT       [2048, 1024] partial
Schedule: two head-pair streams software-pipelined (av/sums trail scores/exp
by 2 steps) so the in-order PE stream never blocks the next exp; V-tail
projections and per-chunk normalize/out-projection work drip into attention
iterations as PE filler while ScalarE stays saturated.
"""

import os
import sys
import types

import numpy as np

B, S, E, H = 4, 2048, 1024, 16
DK = E // H  # 64
HG = H // 2  # heads per core = 8
DG = HG * DK  # 512 projected dims per core
NCORES = 8

TRACE = bool(os.environ.get("TRN_KERNEL_TRACE"))
# matmul-operand dtype: bf16 single-pass PE (fp32 PSUM accumulation) vs
# fp32 operands (PE double-pumps each matmul -> ~2x slower)
MM_DTYPE = os.environ.get("TRN_MM_DTYPE", "bf16")
LAST_EXEC_TIME_NS = None

_cache = {}


def _env_setup():
    import antenv

    if "antenv.axon_hooks" not in sys.modules:
        mod = types.ModuleType("antenv.axon_hooks")
        mod._hook = None
        mod.set_axon_ntff_profile_hook = lambda h: setattr(mod, "_hook", h)
        mod.get_axon_ntff_profile_hook = lambda: mod._hook
        sys.modules["antenv.axon_hooks"] = mod
        antenv.axon_hooks = mod
        try:
            from trn_agent_boot.trn_boot import _ntff_profile_via_ctypes

            mod.set_axon_ntff_profile_hook(
                _ntff_profile_via_ctypes("/opt/axon/libaxon_pjrt.so")
            )
        except Exception:
            pass

    import concourse.bass_utils as bass_utils

    bass_utils.upload_artifacts = lambda tmpdir: tmpdir

    import concourse.tile as tile
    from concourse import mybir
    from concourse.vector_clock import ScopedClock

    if getattr(tile.TileContext, "_wait_split_patched", False):
        return

    MAX_WAITS = 1  # walrus on this image rejects >1 sync wait per instruction

    def _drain_and_barrier_split(self, tick_clock, wait_clock):
        probe = self.nc.sync.drain()
        wait_clock.add_sem_waits(
            probe.ins, ScopedClock({None: tick_clock.global_clock})
        )
        waits = list(probe.ins.sync_info.on_wait)
        if len(waits) > MAX_WAITS:
            num2h = {h.num: h for h in self.sems.allocated().values()}
            probe.ins.sync_info.on_wait = []
            for w in waits:
                self.nc.sync.wait_ge(num2h[w.id], w.wait_value)
            self.nc.sync.drain()
        self.nc.all_engine_barrier()
        popped = self.nc._tile_sem_poison_stack.pop()
        assert popped is self._sem_poison
        self.nc.clear_and_free_semaphores(list(self.sems.allocated().values()))
        self.nc.all_engine_barrier()

    _orig_commit = tile.TileContext._commit_instruction
    _ctr = [0]

    def _commit_split_waits(self, inst, lazy_reg_writes=True):
        si = inst.sync_info
        if (
            si is not None
            and len(si.on_wait) > MAX_WAITS
            and inst.engine != mybir.EngineType.Unassigned
        ):
            waits = list(si.on_wait)
            keep, hoist = waits[:MAX_WAITS], waits[MAX_WAITS:]
            for i in range(0, len(hoist), MAX_WAITS):
                _ctr[0] += 1
                nop = mybir.InstNoOp(name=f"waitnop-{_ctr[0]}", ins=[], outs=[])
                nop.engine = inst.engine
                nop.sync_info = mybir.SyncInfo(
                    on_wait=hoist[i : i + MAX_WAITS], on_update=[]
                )
                self.nc.register_instruction(nop, overwrite=True)
                _orig_commit(self, nop, lazy_reg_writes=False)
            inst.sync_info = mybir.SyncInfo(on_wait=keep, on_update=list(si.on_update))
        return _orig_commit(self, inst, lazy_reg_writes=lazy_reg_writes)

    tile.TileContext._drain_and_barrier = _drain_and_barrier_split
    tile.TileContext._commit_instruction = _commit_split_waits
    tile.TileContext._wait_split_patched = True

    # use the full usable SBUF on trn2 (default constant is stale)
    import concourse.tile_utils as tile_utils

    tile_utils.max_sbuf_usage = 206 * 1024

    if os.environ.get("TRN_LDW_OPT"):
        _orig_bvo = bass_utils.bir_verify_and_optimise

        def _bvo_ldwopt(*a, **kw):
            orig_run = bass_utils.run_command

            def run_patched(cmd, **rkw):
                cmd = [
                    c.replace("--enable-ldw-opt=false", "--enable-ldw-opt=true")
                    if isinstance(c, str)
                    else c
                    for c in cmd
                ]
                return orig_run(cmd, **rkw)

            bass_utils.run_command = run_patched
            try:
                return _orig_bvo(*a, **kw)
            finally:
                bass_utils.run_command = orig_run

        bass_utils.bir_verify_and_optimise = _bvo_ldwopt


def _build_nc():
    import contextlib

    import concourse.bass as bass
    import concourse.tile as tile
    from concourse import mybir

    F32 = mybir.dt.float32
    CDT = mybir.dt.bfloat16 if MM_DTYPE == "bf16" else mybir.dt.float32
    PS = bass.MemorySpace.PSUM
    AF = mybir.ActivationFunctionType

    nc = bass.Bass()
    xT_d = nc.dram_tensor("xT", [E, S], CDT, kind="ExternalInput")
    wqT_d = nc.dram_tensor("wqT", [E, DG], CDT, kind="ExternalInput")
    wkT_d = nc.dram_tensor("wkT", [E, DG], CDT, kind="ExternalInput")
    wvT_d = nc.dram_tensor("wvT", [E, DG], CDT, kind="ExternalInput")
    woT_d = nc.dram_tensor("woT", [DG, E], CDT, kind="ExternalInput")
    sel_d = nc.dram_tensor("sel", [HG, 512], CDT, kind="ExternalInput")
    out_d = nc.dram_tensor("out", [S, E], F32, kind="ExternalOutput")

    NE = E // 128  # 8 e-tiles
    NT = S // 128  # 16 t/s-tiles
    NNC = S // 512  # 4 s-chunks
    NM = DG // 128  # 4 head-pair tiles

    with tile.TileContext(nc) as tc:
        st = contextlib.ExitStack()
        with st:
            pp = st.enter_context(tc.tile_pool(name="persist", bufs=1))
            stg = st.enter_context(tc.tile_pool(name="stage", bufs=12))
            expp = st.enter_context(tc.tile_pool(name="expp", bufs=8))
            outp = st.enter_context(tc.tile_pool(name="outp", bufs=2))

            sel_sb = pp.tile([HG, 512], CDT, tag="sel")
            nc.sync.dma_start(sel_sb[:], sel_d[:])

            QT = pp.tile([128, NM * S], CDT, tag="QT")  # [128, 8192]
            KT = pp.tile([128, NM * S], CDT, tag="KT")
            Vsb = pp.tile([128, NT * DG], CDT, tag="V")  # [128, 8192]
            onorm = pp.tile([128, NM * S], CDT, tag="onorm")
            zbuf = pp.tile([HG, S], F32, tag="zbuf")
            zrec = pp.tile([HG, S], F32, tag="zrec")
            zrecc = pp.tile([HG, S], CDT, tag="zrecc")
            zstage = pp.tile([128, NM * 512], F32, tag="zstage")
            woT = pp.tile([128, NM * E], CDT, tag="woT")  # [128, 4096]
            ones = pp.tile([128, 1], CDT, tag="ones")
            nc.gpsimd.memset(ones[:], 1.0)

            xT = pp.tile([128, NE * S], CDT, tag="xT")  # [128, 16384]
            wq = pp.tile([128, NE * DG], CDT, tag="wq")
            wk = pp.tile([128, NE * DG], CDT, tag="wk")
            wv = pp.tile([128, NE * DG], CDT, tag="wv")
            for j in range(NE):
                nc.sync.dma_start(
                    xT[:, j * S : (j + 1) * S], xT_d[j * 128 : (j + 1) * 128, :]
                )
            for w_sb, w_d in ((wq, wqT_d), (wk, wkT_d), (wv, wvT_d)):
                for j in range(NE):
                    nc.sync.dma_start(
                        w_sb[:, j * DG : (j + 1) * DG],
                        w_d[j * 128 : (j + 1) * 128, :],
                    )
            for k in range(NM):
                nc.sync.dma_start(
                    woT[:, k * E : (k + 1) * E], woT_d[k * 128 : (k + 1) * 128, :]
                )

            def qk_proj(w_sb, dst, m, n):
                acc = proj_ps.tile([128, 512], F32, tag="acc")
                for j in range(NE):
                    nc.tensor.matmul(
                        acc[:],
                        w_sb[:, j * DG + m * 128 : j * DG + (m + 1) * 128],
                        xT[:, j * S + n * 512 : j * S + (n + 1) * 512],
                        start=(j == 0),
                        stop=(j == NE - 1),
                    )
                nc.vector.tensor_copy(
                    dst[:, m * S + n * 512 : m * S + (n + 1) * 512], acc[:]
                )

            def v_proj(i):
                acc = proj_ps.tile([128, 512], F32, tag="acc")
                for j in range(NE):
                    nc.tensor.matmul(
                        acc[:],
                        xT[:, j * S + i * 128 : j * S + (i + 1) * 128],
                        wv[:, j * DG : (j + 1) * DG],
                        start=(j == 0),
                        stop=(j == NE - 1),
                    )
                nc.vector.tensor_copy(Vsb[:, i * DG : (i + 1) * DG], acc[:])

            # projection phase under a scoped, deep psum pool; the V-tail
            # drips into the first attention group (consumed two steps later)
            with tc.tile_pool(name="projps", bufs=3, space=PS) as proj_ps:
                for m in range(NM):
                    for n in range(NNC):
                        qk_proj(wq, QT, m, n)
                        qk_proj(wk, KT, m, n)
                for i in range(4):
                    v_proj(i)

            sc_ps = st.enter_context(tc.tile_pool(name="scpsum", bufs=2, space=PS))
            av_ps = st.enter_context(tc.tile_pool(name="avpsum", bufs=2, space=PS))
            z_ps = st.enter_context(tc.tile_pool(name="zpsum", bufs=1, space=PS))
            ms_ps = st.enter_context(tc.tile_pool(name="miscpsum", bufs=1, space=PS))

            filler = []

            def v_proj_ms(i):
                acc = ms_ps.tile([128, 512], F32, tag="ms")
                for j in range(NE):
                    nc.tensor.matmul(
                        acc[:],
                        xT[:, j * S + i * 128 : j * S + (i + 1) * 128],
                        wv[:, j * DG : (j + 1) * DG],
                        start=(j == 0),
                        stop=(j == NE - 1),
                    )
                nc.vector.tensor_copy(Vsb[:, i * DG : (i + 1) * DG], acc[:])

            for i in range(4, NT):
                filler.append(lambda i=i: v_proj_ms(i))

            def drip():
                if filler:
                    filler.pop(0)()

            def score_step(hp, n, t):
                sc = sc_ps.tile([128, 1024], F32, tag="sc")
                nc.tensor.matmul(
                    sc[:, 0:512],
                    KT[0:64, hp * S + t * 128 : hp * S + (t + 1) * 128],
                    QT[0:64, hp * S + n * 512 : hp * S + (n + 1) * 512],
                )
                nc.tensor.matmul(
                    sc[:, 512:1024],
                    KT[64:128, hp * S + t * 128 : hp * S + (t + 1) * 128],
                    QT[64:128, hp * S + n * 512 : hp * S + (n + 1) * 512],
                )
                ex = expp.tile([128, 1024], CDT, tag="ex")
                nc.scalar.activation(ex[:], sc[:], AF.Exp, scale=0.125)
                return ex

            def av_pair(hp, t, av, ex):
                voff = t * DG
                nc.tensor.matmul(
                    av[0:64, :],
                    Vsb[:, voff + (2 * hp) * DK : voff + (2 * hp) * DK + DK],
                    ex[:, 0:512],
                    start=(t == 0),
                    stop=(t == NT - 1),
                    tile_position=(0, 0),
                    skip_group_check=True,
                )
                nc.tensor.matmul(
                    av[64:128, :],
                    Vsb[:, voff + (2 * hp + 1) * DK : voff + (2 * hp + 1) * DK + DK],
                    ex[:, 512:1024],
                    start=(t == 0),
                    stop=(t == NT - 1),
                    tile_position=(0, 64),
                    skip_group_check=True,
                )

            def sum_pair(t, zz, zrow, ex):
                nc.tensor.matmul(
                    zz[zrow : zrow + 1, :],
                    ones[:, 0:1],
                    ex[:, 0:512],
                    start=(t == 0),
                    stop=(t == NT - 1),
                    tile_position=(0, zrow),
                    skip_group_check=True,
                )
                nc.tensor.matmul(
                    zz[zrow + 32 : zrow + 33, :],
                    ones[:, 0:1],
                    ex[:, 512:1024],
                    start=(t == 0),
                    stop=(t == NT - 1),
                    tile_position=(0, zrow + 32),
                    skip_group_check=True,
                )

            def boundary(n):
                """post-chunk-n work, dripped into chunk n+1: normalize+outproj."""
                tasks = []
                for k in range(NM):
                    def norm_k(k=k, n=n):
                        rep = ms_ps.tile([128, 512], F32, tag="ms")
                        nc.tensor.matmul(
                            rep[:],
                            sel_sb[:, k * 128 : (k + 1) * 128],
                            zrecc[:, n * 512 : (n + 1) * 512],
                        )
                        nc.vector.tensor_tensor(
                            onorm[:, k * S + n * 512 : k * S + (n + 1) * 512],
                            uos[(n, k)][:],
                            rep[:],
                            mybir.AluOpType.mult,
                        )
                    tasks.append(norm_k)
                osbs = {}
                for i in range(4 * n, 4 * n + 4):
                    def oproj_pre(i=i):
                        osbs[i] = outp.tile([128, E], F32, tag="osb", name=f"osb{i}")
                    def oproj_eh(i=i, eh=0):
                        ps = ms_ps.tile([128, 512], F32, tag="ms")
                        for k in range(NM):
                            nc.tensor.matmul(
                                ps[:],
                                onorm[:, k * S + i * 128 : k * S + (i + 1) * 128],
                                woT[:, k * E + eh * 512 : k * E + (eh + 1) * 512],
                                start=(k == 0),
                                stop=(k == NM - 1),
                            )
                        nc.vector.tensor_copy(
                            osbs[i][:, eh * 512 : (eh + 1) * 512], ps[:]
                        )
                    def oproj_out(i=i):
                        nc.sync.dma_start(
                            out_d[i * 128 : (i + 1) * 128, :], osbs[i][:]
                        )
                    tasks.append(oproj_pre)
                    tasks.append(lambda i=i: oproj_eh(i, 0))
                    tasks.append(lambda i=i: oproj_eh(i, 1))
                    tasks.append(oproj_out)
                return tasks

            uos = {}
            for n in range(NNC):
                for hpp in (0, 2):
                    av_a = av_ps.tile([128, 512], F32, tag="av")
                    av_b = av_ps.tile([128, 512], F32, tag="av")
                    zz = z_ps.tile([128, 512], F32, tag="zz")
                    DLY = 2  # av/sums trail scores/exp by 2 steps
                    pend = []
                    for t in range(NT + DLY):
                        if t < NT:
                            ex_a = score_step(hpp, n, t)
                            ex_b = score_step(hpp + 1, n, t)
                            pend.append((t, ex_a, ex_b))
                        if len(pend) > DLY or t >= NT:
                            pt, pa, pb = pend.pop(0)
                            av_pair(hpp, pt, av_a, pa)
                            av_pair(hpp + 1, pt, av_b, pb)
                            sum_pair(pt, zz, 0, pa)
                            sum_pair(pt, zz, 64, pb)
                        if 4 <= t < NT:
                            drip()
                    for hp, av, zrow in ((hpp, av_a, 0), (hpp + 1, av_b, 64)):
                        uo = stg.tile([128, 512], F32, tag="uo")
                        nc.vector.tensor_copy(uo[:], av[:])
                        nc.vector.tensor_copy(
                            zstage[0:1, hp * 512 : (hp + 1) * 512],
                            zz[zrow : zrow + 1, :],
                        )
                        nc.vector.tensor_copy(
                            zstage[32:33, hp * 512 : (hp + 1) * 512],
                            zz[zrow + 32 : zrow + 33, :],
                        )
                        uos[(n, hp)] = uo

                # zbuf rows 0-3 = even heads of pair hp, rows 4-7 = odd heads
                for hp in range(NM):
                    nc.sync.dma_start(
                        zbuf[hp : hp + 1, n * 512 : (n + 1) * 512],
                        zstage[0:1, hp * 512 : (hp + 1) * 512],
                    )
                    nc.sync.dma_start(
                        zbuf[4 + hp : 5 + hp, n * 512 : (n + 1) * 512],
                        zstage[32:33, hp * 512 : (hp + 1) * 512],
                    )
                nc.vector.reciprocal(
                    zrec[:, n * 512 : (n + 1) * 512], zbuf[:, n * 512 : (n + 1) * 512]
                )
                nc.vector.tensor_copy(
                    zrecc[:, n * 512 : (n + 1) * 512], zrec[:, n * 512 : (n + 1) * 512]
                )
                filler.extend(boundary(n))
            while filler:
                filler.pop(0)()

    return nc


def _make_sel():
    # zbuf row for head (2k + p//64): even heads -> row k, odd heads -> row 4+k
    sel = np.zeros((HG, 512), dtype=np.float32)
    for k in range(4):
        for p in range(128):
            r = k if p < 64 else 4 + k
            sel[r, k * 128 + p] = 1.0
    return sel


def kernel(x, Wq, Wk, Wv, Wo):
    global LAST_EXEC_TIME_NS
    _env_setup()
    from concourse.bass_utils import run_bass_kernel_spmd

    x = np.asarray(x, dtype=np.float32)
    Wq = np.asarray(Wq, dtype=np.float32)
    Wk = np.asarray(Wk, dtype=np.float32)
    Wv = np.asarray(Wv, dtype=np.float32)
    Wo = np.asarray(Wo, dtype=np.float32)

    if "nc" not in _cache:
        _cache["nc"] = _build_nc()
    nc = _cache["nc"]

    if MM_DTYPE == "bf16":
        import ml_dtypes

        cdt = ml_dtypes.bfloat16
    else:
        cdt = np.float32

    sel = _make_sel()
    in_maps = []
    for c in range(NCORES):
        b, hg = c // 2, c % 2
        r = slice(hg * DG, (hg + 1) * DG)
        in_maps.append(
            {
                "xT": np.ascontiguousarray(x[b].T).astype(cdt),
                "wqT": np.ascontiguousarray(Wq[r, :].T).astype(cdt),
                "wkT": np.ascontiguousarray(Wk[r, :].T).astype(cdt),
                "wvT": np.ascontiguousarray(Wv[r, :].T).astype(cdt),
                "woT": np.ascontiguousarray(Wo[:, r].T).astype(cdt),
                "sel": sel.astype(cdt),
            }
        )

    res = run_bass_kernel_spmd(
        nc, in_maps, core_ids=list(range(NCORES)), trace=TRACE
    )
    if TRACE:
        LAST_EXEC_TIME_NS = res.exec_time_ns

    out = np.empty((B, S, E), dtype=np.float32)
    for b in range(B):
        out[b] = res.results[2 * b]["out"] + res.results[2 * b + 1]["out"]
    return out


# revision 31
# speedup vs baseline: 1.5169x; 1.0040x over previous
"""Multi-head attention Trainium2 kernel (8 NeuronCores, Bass/Tile).

Sharding: core c -> (batch b = c//2, head-group hg = c%2). Each core computes
attention for 8 of the 16 heads of one batch element plus its partial
out-projection; the host sums the two head-group partials per batch.

Per-core layouts (host pre-transposes inputs; contraction dims on partitions):
  xT  [E=1024, S=2048]      x[b].T
  wqT/wkT/wvT [1024, 512]   W[hg_rows].T
  woT [512, 1024]           Wo[:, hg_cols].T
  sel [8, 512]              0/1 selector for softmax-denominator replication

On-chip pipeline (bf16 matmul operands, fp32 PSUM accumulation/softmax):
  QT = wqT.T-tiles @ xT   [512, 2048] (head-major, transposed)
  KT likewise; V natural [2048, 512]
  scoresT[t,s] = KT_h.T-tile @ QT_h   (K=64; two heads row-packed -> concurrent)
  expT = exp(scoresT/8) on ScalarE    ([128,1024] psum->sbuf per t-tile)
  outT = V_h.T @ expT                 (col-packed pair -> concurrent)
  Z    = ones.T @ expT                (col-tiled M=1 pair -> concurrent)
  outT_norm = outT * replicate(1/Z)   (K=8 selector matmul + DVE mult)
  out = outT_norm.T-tiles @ woT       [2048, 1024] partial
Schedule: two head-pair streams software-pipelined (av/sums trail scores/exp
by 2 steps) so the in-order PE stream never blocks the next exp; V-tail
projections and per-chunk normalize/out-projection work drip into attention
iterations as PE filler while ScalarE stays saturated.
"""

import os
import sys
import types

import numpy as np

B, S, E, H = 4, 2048, 1024, 16
DK = E // H  # 64
HG = H // 2  # heads per core = 8
DG = HG * DK  # 512 projected dims per core
NCORES = 8

TRACE = bool(os.environ.get("TRN_KERNEL_TRACE"))
# matmul-operand dtype: bf16 single-pass PE (fp32 PSUM accumulation) vs
# fp32 operands (PE double-pumps each matmul -> ~2x slower)
MM_DTYPE = os.environ.get("TRN_MM_DTYPE", "bf16")
LAST_EXEC_TIME_NS = None

_cache = {}


def _env_setup():
    import antenv

    if "antenv.axon_hooks" not in sys.modules:
        mod = types.ModuleType("antenv.axon_hooks")
        mod._hook = None
        mod.set_axon_ntff_profile_hook = lambda h: setattr(mod, "_hook", h)
        mod.get_axon_ntff_profile_hook = lambda: mod._hook
        sys.modules["antenv.axon_hooks"] = mod
        antenv.axon_hooks = mod
        try:
            from trn_agent_boot.trn_boot import _ntff_profile_via_ctypes

            mod.set_axon_ntff_profile_hook(
                _ntff_profile_via_ctypes("/opt/axon/libaxon_pjrt.so")
            )
        except Exception:
            pass

    import concourse.bass_utils as bass_utils

    bass_utils.upload_artifacts = lambda tmpdir: tmpdir

    import concourse.tile as tile
    from concourse import mybir
    from concourse.vector_clock import ScopedClock

    if getattr(tile.TileContext, "_wait_split_patched", False):
        return

    MAX_WAITS = 1  # walrus on this image rejects >1 sync wait per instruction

    def _drain_and_barrier_split(self, tick_clock, wait_clock):
        probe = self.nc.sync.drain()
        wait_clock.add_sem_waits(
            probe.ins, ScopedClock({None: tick_clock.global_clock})
        )
        waits = list(probe.ins.sync_info.on_wait)
        if len(waits) > MAX_WAITS:
            num2h = {h.num: h for h in self.sems.allocated().values()}
            probe.ins.sync_info.on_wait = []
            for w in waits:
                self.nc.sync.wait_ge(num2h[w.id], w.wait_value)
            self.nc.sync.drain()
        self.nc.all_engine_barrier()
        popped = self.nc._tile_sem_poison_stack.pop()
        assert popped is self._sem_poison
        self.nc.clear_and_free_semaphores(list(self.sems.allocated().values()))
        self.nc.all_engine_barrier()

    _orig_commit = tile.TileContext._commit_instruction
    _ctr = [0]

    def _commit_split_waits(self, inst, lazy_reg_writes=True):
        si = inst.sync_info
        if (
            si is not None
            and len(si.on_wait) > MAX_WAITS
            and inst.engine != mybir.EngineType.Unassigned
        ):
            waits = list(si.on_wait)
            keep, hoist = waits[:MAX_WAITS], waits[MAX_WAITS:]
            for i in range(0, len(hoist), MAX_WAITS):
                _ctr[0] += 1
                nop = mybir.InstNoOp(name=f"waitnop-{_ctr[0]}", ins=[], outs=[])
                nop.engine = inst.engine
                nop.sync_info = mybir.SyncInfo(
                    on_wait=hoist[i : i + MAX_WAITS], on_update=[]
                )
                self.nc.register_instruction(nop, overwrite=True)
                _orig_commit(self, nop, lazy_reg_writes=False)
            inst.sync_info = mybir.SyncInfo(on_wait=keep, on_update=list(si.on_update))
        return _orig_commit(self, inst, lazy_reg_writes=lazy_reg_writes)

    tile.TileContext._drain_and_barrier = _drain_and_barrier_split
    tile.TileContext._commit_instruction = _commit_split_waits
    tile.TileContext._wait_split_patched = True

    # use the full usable SBUF on trn2 (default constant is stale)
    import concourse.tile_utils as tile_utils

    tile_utils.max_sbuf_usage = 206 * 1024

    if os.environ.get("TRN_LDW_OPT"):
        _orig_bvo = bass_utils.bir_verify_and_optimise

        def _bvo_ldwopt(*a, **kw):
            orig_run = bass_utils.run_command

            def run_patched(cmd, **rkw):
                cmd = [
                    c.replace("--enable-ldw-opt=false", "--enable-ldw-opt=true")
                    if isinstance(c, str)
                    else c
                    for c in cmd
                ]
                return orig_run(cmd, **rkw)

            bass_utils.run_command = run_patched
            try:
                return _orig_bvo(*a, **kw)
            finally:
                bass_utils.run_command = orig_run

        bass_utils.bir_verify_and_optimise = _bvo_ldwopt


def _build_nc():
    import contextlib

    import concourse.bass as bass
    import concourse.tile as tile
    from concourse import mybir

    F32 = mybir.dt.float32
    CDT = mybir.dt.bfloat16 if MM_DTYPE == "bf16" else mybir.dt.float32
    PS = bass.MemorySpace.PSUM
    AF = mybir.ActivationFunctionType

    nc = bass.Bass()
    xT_d = nc.dram_tensor("xT", [E, S], CDT, kind="ExternalInput")
    wqT_d = nc.dram_tensor("wqT", [E, DG], CDT, kind="ExternalInput")
    wkT_d = nc.dram_tensor("wkT", [E, DG], CDT, kind="ExternalInput")
    wvT_d = nc.dram_tensor("wvT", [E, DG], CDT, kind="ExternalInput")
    woT_d = nc.dram_tensor("woT", [DG, E], CDT, kind="ExternalInput")
    sel_d = nc.dram_tensor("sel", [HG, 512], CDT, kind="ExternalInput")
    out_d = nc.dram_tensor("out", [S, E], F32, kind="ExternalOutput")

    NE = E // 128  # 8 e-tiles
    NT = S // 128  # 16 t/s-tiles
    NNC = S // 512  # 4 s-chunks
    NM = DG // 128  # 4 head-pair tiles

    with tile.TileContext(nc) as tc:
        st = contextlib.ExitStack()
        with st:
            pp = st.enter_context(tc.tile_pool(name="persist", bufs=1))
            stg = st.enter_context(tc.tile_pool(name="stage", bufs=12))
            expp = st.enter_context(tc.tile_pool(name="expp", bufs=8))
            outp = st.enter_context(tc.tile_pool(name="outp", bufs=2))

            sel_sb = pp.tile([HG, 512], CDT, tag="sel")
            nc.sync.dma_start(sel_sb[:], sel_d[:])

            QT = pp.tile([128, NM * S], CDT, tag="QT")  # [128, 8192]
            KT = pp.tile([128, NM * S], CDT, tag="KT")
            Vsb = pp.tile([128, NT * DG], CDT, tag="V")  # [128, 8192]
            onorm = pp.tile([128, NM * S], CDT, tag="onorm")
            zbuf = pp.tile([HG, S], F32, tag="zbuf")
            zrec = pp.tile([HG, S], F32, tag="zrec")
            zrecc = pp.tile([HG, S], CDT, tag="zrecc")
            zstage = pp.tile([128, NM * 512], F32, tag="zstage")
            woT = pp.tile([128, NM * E], CDT, tag="woT")  # [128, 4096]
            ones = pp.tile([128, 1], CDT, tag="ones")
            nc.gpsimd.memset(ones[:], 1.0)

            xT = pp.tile([128, NE * S], CDT, tag="xT")  # [128, 16384]
            wq = pp.tile([128, NE * DG], CDT, tag="wq")
            wk = pp.tile([128, NE * DG], CDT, tag="wk")
            wv = pp.tile([128, NE * DG], CDT, tag="wv")
            for j in range(NE):
                nc.sync.dma_start(
                    xT[:, j * S : (j + 1) * S], xT_d[j * 128 : (j + 1) * 128, :]
                )
            for w_sb, w_d in ((wq, wqT_d), (wk, wkT_d), (wv, wvT_d)):
                for j in range(NE):
                    nc.sync.dma_start(
                        w_sb[:, j * DG : (j + 1) * DG],
                        w_d[j * 128 : (j + 1) * 128, :],
                    )
            for k in range(NM):
                nc.sync.dma_start(
                    woT[:, k * E : (k + 1) * E], woT_d[k * 128 : (k + 1) * 128, :]
                )

            def qk_proj(w_sb, dst, m, n):
                acc = proj_ps.tile([128, 512], F32, tag="acc")
                for j in range(NE):
                    nc.tensor.matmul(
                        acc[:],
                        w_sb[:, j * DG + m * 128 : j * DG + (m + 1) * 128],
                        xT[:, j * S + n * 512 : j * S + (n + 1) * 512],
                        start=(j == 0),
                        stop=(j == NE - 1),
                    )
                nc.vector.tensor_copy(
                    dst[:, m * S + n * 512 : m * S + (n + 1) * 512], acc[:]
                )

            def v_proj(i):
                acc = proj_ps.tile([128, 512], F32, tag="acc")
                for j in range(NE):
                    nc.tensor.matmul(
                        acc[:],
                        xT[:, j * S + i * 128 : j * S + (i + 1) * 128],
                        wv[:, j * DG : (j + 1) * DG],
                        start=(j == 0),
                        stop=(j == NE - 1),
                    )
                nc.vector.tensor_copy(Vsb[:, i * DG : (i + 1) * DG], acc[:])

            # projection phase under a scoped, deep psum pool; the V-tail
            # drips into the first attention group (consumed two steps later)
            with tc.tile_pool(name="projps", bufs=3, space=PS) as proj_ps:
                for m in (0, 1):
                    for n in range(NNC):
                        qk_proj(wq, QT, m, n)
                        qk_proj(wk, KT, m, n)
                for i in range(4):
                    v_proj(i)

            sc_ps = st.enter_context(tc.tile_pool(name="scpsum", bufs=2, space=PS))
            av_ps = st.enter_context(tc.tile_pool(name="avpsum", bufs=2, space=PS))
            z_ps = st.enter_context(tc.tile_pool(name="zpsum", bufs=1, space=PS))
            ms_ps = st.enter_context(tc.tile_pool(name="miscpsum", bufs=1, space=PS))

            filler = []

            def v_proj_ms(i):
                acc = ms_ps.tile([128, 512], F32, tag="ms")
                for j in range(NE):
                    nc.tensor.matmul(
                        acc[:],
                        xT[:, j * S + i * 128 : j * S + (i + 1) * 128],
                        wv[:, j * DG : (j + 1) * DG],
                        start=(j == 0),
                        stop=(j == NE - 1),
                    )
                nc.vector.tensor_copy(Vsb[:, i * DG : (i + 1) * DG], acc[:])

            def qk_proj_ms(w_sb, dst, m, n):
                acc = ms_ps.tile([128, 512], F32, tag="ms")
                for j in range(NE):
                    nc.tensor.matmul(
                        acc[:],
                        w_sb[:, j * DG + m * 128 : j * DG + (m + 1) * 128],
                        xT[:, j * S + n * 512 : j * S + (n + 1) * 512],
                        start=(j == 0),
                        stop=(j == NE - 1),
                    )
                nc.vector.tensor_copy(
                    dst[:, m * S + n * 512 : m * S + (n + 1) * 512], acc[:]
                )

            for i in range(4, NT):
                filler.append(lambda i=i: v_proj_ms(i))
            for m in (2, 3):
                for n in range(NNC):
                    filler.append(lambda m=m, n=n: qk_proj_ms(wq, QT, m, n))
                    filler.append(lambda m=m, n=n: qk_proj_ms(wk, KT, m, n))

            def drip():
                if filler:
                    filler.pop(0)()

            def score_step(hp, n, t):
                sc = sc_ps.tile([128, 1024], F32, tag="sc")
                nc.tensor.matmul(
                    sc[:, 0:512],
                    KT[0:64, hp * S + t * 128 : hp * S + (t + 1) * 128],
                    QT[0:64, hp * S + n * 512 : hp * S + (n + 1) * 512],
                )
                nc.tensor.matmul(
                    sc[:, 512:1024],
                    KT[64:128, hp * S + t * 128 : hp * S + (t + 1) * 128],
                    QT[64:128, hp * S + n * 512 : hp * S + (n + 1) * 512],
                )
                ex = expp.tile([128, 1024], CDT, tag="ex")
                nc.scalar.activation(ex[:], sc[:], AF.Exp, scale=0.125)
                return ex

            def av_pair(hp, t, av, ex):
                voff = t * DG
                nc.tensor.matmul(
                    av[0:64, :],
                    Vsb[:, voff + (2 * hp) * DK : voff + (2 * hp) * DK + DK],
                    ex[:, 0:512],
                    start=(t == 0),
                    stop=(t == NT - 1),
                    tile_position=(0, 0),
                    skip_group_check=True,
                )
                nc.tensor.matmul(
                    av[64:128, :],
                    Vsb[:, voff + (2 * hp + 1) * DK : voff + (2 * hp + 1) * DK + DK],
                    ex[:, 512:1024],
                    start=(t == 0),
                    stop=(t == NT - 1),
                    tile_position=(0, 64),
                    skip_group_check=True,
                )

            def sum_pair(t, zz, zrow, ex):
                nc.tensor.matmul(
                    zz[zrow : zrow + 1, :],
                    ones[:, 0:1],
                    ex[:, 0:512],
                    start=(t == 0),
                    stop=(t == NT - 1),
                    tile_position=(0, zrow),
                    skip_group_check=True,
                )
                nc.tensor.matmul(
                    zz[zrow + 32 : zrow + 33, :],
                    ones[:, 0:1],
                    ex[:, 512:1024],
                    start=(t == 0),
                    stop=(t == NT - 1),
                    tile_position=(0, zrow + 32),
                    skip_group_check=True,
                )

            def boundary(n):
                """post-chunk-n work, dripped into chunk n+1: normalize+outproj."""
                tasks = []
                for k in range(NM):
                    def norm_k(k=k, n=n):
                        rep = ms_ps.tile([128, 512], F32, tag="ms")
                        nc.tensor.matmul(
                            rep[:],
                            sel_sb[:, k * 128 : (k + 1) * 128],
                            zrecc[:, n * 512 : (n + 1) * 512],
                        )
                        nc.vector.tensor_tensor(
                            onorm[:, k * S + n * 512 : k * S + (n + 1) * 512],
                            uos[(n, k)][:],
                            rep[:],
                            mybir.AluOpType.mult,
                        )
                    tasks.append(norm_k)
                osbs = {}
                for i in range(4 * n, 4 * n + 4):
                    def oproj_pre(i=i):
                        osbs[i] = outp.tile([128, E], F32, tag="osb", name=f"osb{i}")
                    def oproj_eh(i=i, eh=0):
                        ps = ms_ps.tile([128, 512], F32, tag="ms")
                        for k in range(NM):
                            nc.tensor.matmul(
                                ps[:],
                                onorm[:, k * S + i * 128 : k * S + (i + 1) * 128],
                                woT[:, k * E + eh * 512 : k * E + (eh + 1) * 512],
                                start=(k == 0),
                                stop=(k == NM - 1),
                            )
                        nc.vector.tensor_copy(
                            osbs[i][:, eh * 512 : (eh + 1) * 512], ps[:]
                        )
                    def oproj_out(i=i):
                        nc.sync.dma_start(
                            out_d[i * 128 : (i + 1) * 128, :], osbs[i][:]
                        )
                    tasks.append(oproj_pre)
                    tasks.append(lambda i=i: oproj_eh(i, 0))
                    tasks.append(lambda i=i: oproj_eh(i, 1))
                    tasks.append(oproj_out)
                return tasks

            uos = {}
            done = {n: 0 for n in range(NNC)}
            for n, hpp in (
                (0, 0), (1, 0), (2, 0), (0, 2), (3, 0), (1, 2), (2, 2), (3, 2)
            ):
                if True:
                    av_a = av_ps.tile([128, 512], F32, tag="av")
                    av_b = av_ps.tile([128, 512], F32, tag="av")
                    zz = z_ps.tile([128, 512], F32, tag="zz")
                    DLY = 2  # av/sums trail scores/exp by 2 steps
                    pend = []
                    for t in range(NT + DLY):
                        if t < NT:
                            ex_a = score_step(hpp, n, t)
                            ex_b = score_step(hpp + 1, n, t)
                            pend.append((t, ex_a, ex_b))
                        if len(pend) > DLY or t >= NT:
                            pt, pa, pb = pend.pop(0)
                            av_pair(hpp, pt, av_a, pa)
                            av_pair(hpp + 1, pt, av_b, pb)
                            sum_pair(pt, zz, 0, pa)
                            sum_pair(pt, zz, 64, pb)
                        if 4 <= t < NT:
                            drip()
                    for hp, av, zrow in ((hpp, av_a, 0), (hpp + 1, av_b, 64)):
                        uo = stg.tile([128, 512], F32, tag="uo")
                        nc.vector.tensor_copy(uo[:], av[:])
                        nc.vector.tensor_copy(
                            zstage[0:1, hp * 512 : (hp + 1) * 512],
                            zz[zrow : zrow + 1, :],
                        )
                        nc.vector.tensor_copy(
                            zstage[32:33, hp * 512 : (hp + 1) * 512],
                            zz[zrow + 32 : zrow + 33, :],
                        )
                        uos[(n, hp)] = uo

                # zbuf rows 0-3 = even heads of pair hp, rows 4-7 = odd heads
                for hp in (hpp, hpp + 1):
                    nc.sync.dma_start(
                        zbuf[hp : hp + 1, n * 512 : (n + 1) * 512],
                        zstage[0:1, hp * 512 : (hp + 1) * 512],
                    )
                    nc.sync.dma_start(
                        zbuf[4 + hp : 5 + hp, n * 512 : (n + 1) * 512],
                        zstage[32:33, hp * 512 : (hp + 1) * 512],
                    )
                done[n] += 1
                if done[n] == 2:
                    nc.vector.reciprocal(
                        zrec[:, n * 512 : (n + 1) * 512],
                        zbuf[:, n * 512 : (n + 1) * 512],
                    )
                    nc.vector.tensor_copy(
                        zrecc[:, n * 512 : (n + 1) * 512],
                        zrec[:, n * 512 : (n + 1) * 512],
                    )
                    filler.extend(boundary(n))
            while filler:
                filler.pop(0)()

    return nc


def _make_sel():
    # zbuf row for head (2k + p//64): even heads -> row k, odd heads -> row 4+k
    sel = np.zeros((HG, 512), dtype=np.float32)
    for k in range(4):
        for p in range(128):
            r = k if p < 64 else 4 + k
            sel[r, k * 128 + p] = 1.0
    return sel


def kernel(x, Wq, Wk, Wv, Wo):
    global LAST_EXEC_TIME_NS
    _env_setup()
    from concourse.bass_utils import run_bass_kernel_spmd

    x = np.asarray(x, dtype=np.float32)
    Wq = np.asarray(Wq, dtype=np.float32)
    Wk = np.asarray(Wk, dtype=np.float32)
    Wv = np.asarray(Wv, dtype=np.float32)
    Wo = np.asarray(Wo, dtype=np.float32)

    if "nc" not in _cache:
        _cache["nc"] = _build_nc()
    nc = _cache["nc"]

    if MM_DTYPE == "bf16":
        import ml_dtypes

        cdt = ml_dtypes.bfloat16
    else:
        cdt = np.float32

    sel = _make_sel()
    in_maps = []
    for c in range(NCORES):
        b, hg = c // 2, c % 2
        r = slice(hg * DG, (hg + 1) * DG)
        in_maps.append(
            {
                "xT": np.ascontiguousarray(x[b].T).astype(cdt),
                "wqT": np.ascontiguousarray(Wq[r, :].T).astype(cdt),
                "wkT": np.ascontiguousarray(Wk[r, :].T).astype(cdt),
                "wvT": np.ascontiguousarray(Wv[r, :].T).astype(cdt),
                "woT": np.ascontiguousarray(Wo[:, r].T).astype(cdt),
                "sel": sel.astype(cdt),
            }
        )

    res = run_bass_kernel_spmd(
        nc, in_maps, core_ids=list(range(NCORES)), trace=TRACE
    )
    if TRACE:
        LAST_EXEC_TIME_NS = res.exec_time_ns

    out = np.empty((B, S, E), dtype=np.float32)
    for b in range(B):
        out[b] = res.results[2 * b]["out"] + res.results[2 * b + 1]["out"]
    return out
